# revision 1
# baseline (speedup 1.0000x reference)
"""Trainium2 Bass kernel for nn_Dwtpool (dense_cnn).

Reference graph (per image, C=256, 128x128 input):
  p    = maxpool2x2(x)                          -> [256, 64, 64]
  r    = ReLU(BN(conv1x1(x, reduce_w)))         -> [ 64,128,128]
  M    = haar_dwt(r) * 2  (stored unscaled)     -> [256, 64, 64]
  q1..q4 = conv{1,3,5,7}(0.5*M)                 -> [256, 64, 64] each
  qkv  = conv3x3(concat[0.5*M, q1..q4, p])      -> [256, 64, 64]
  att  = softmax_spatial(conv1x1(qkv)); pooled = sum_n qkv_ch * att
  cw   = ct2(ReLU(LN(ct1(pooled))))             -> [256]
  out  = conv1x1(qkv * cw, proj_w)              -> [256, 64, 64]

Strategy: data-parallel over batch (16 images / 8 cores = 2 per core).
Channels live on partitions (2 tiles of 128); spatial row-major on the free
dim.  All convs are tap-accumulated bf16 matmuls into PSUM (fp32 accum);
weights stream through a small SBUF ring, one [128, 256] tile per
(ktile, tap), with 8 N=512 matmuls per weight load.  Activations stay in
SBUF in zero-padded bf16 buffers so every conv tap is a plain offset read.
BN is folded into the reduce conv, the DWT 0.5 into downstream weights, the
softmax denominator into the e-broadcast, and the attention channel scale cw
into the proj weights.  Content logits are computed in fp32 from fp32-staged
concat-conv chunks; softmax lives on partition 0; attention pooling is a
fused multiply-reduce against e broadcast to all partitions by a ones-matmul.
"""
import os
import sys

for _p in ("/opt/trn_rl_repo", os.path.expanduser("~/.axon_site/_ro/trn_rl_repo")):
    if os.path.isdir(_p) and _p not in sys.path:
        sys.path.append(_p)

import numpy as np
import ml_dtypes
from contextlib import ExitStack

import concourse.bass as bass
import concourse.tile as tile
from concourse import mybir, bass_isa
from concourse import bass_utils

BF16 = mybir.dt.bfloat16
F32 = mybir.dt.float32
AF = mybir.ActivationFunctionType

B, C, H, W = 16, 256, 128, 128
DQ, DS = 64, 32
H2, W2 = 64, 64
N_CORES = 8
BPC = B // N_CORES  # images per core
EPS = 1e-5

# ---------------------------------------------------------------------------
# walrus CoreV3 rejects instructions with more than a couple of sync waits;
# Tile's exit drain accumulates one wait per processor used.  Split the waits
# across a chain of drain instructions (sync engine executes them in order).
# ---------------------------------------------------------------------------
import bass_rust as _br
import concourse.tile as _tile_mod

def _split_drain_and_barrier(self, tick_clock, wait_clock):
    nc = self.nc
    drain_inst = nc.sync.drain()
    wait_clock.add_sem_waits(
        drain_inst.ins, _tile_mod.ScopedClock({None: tick_clock.global_clock})
    )
    W_ = list(drain_inst.ins.sync_info.on_wait)
    if len(W_) > 1:
        drain_inst.ins.sync_info.on_wait = W_[:1]
        for i in range(1, len(W_)):
            extra = nc.sync.drain()
            extra.ins.sync_info = _br.SyncInfo(on_wait=W_[i : i + 1], on_update=[])
    nc.all_engine_barrier()
    assert self.sems is not None
    popped = nc._tile_sem_poison_stack.pop()
    assert popped is self._sem_poison
    nc.clear_and_free_semaphores(list(self.sems.allocated().values()))
    nc.all_engine_barrier()

tile.TileContext._drain_and_barrier = _split_drain_and_barrier

# Same hardware limit applies to scheduled body instructions (max 2 sync waits
# per instruction).  Before lowering, move excess waits onto injected NOPs on
# the same engine.
_MAX_W = 1
_orig_lower_ordered = tile.TileContext._lower_ordered_insts

def _lower_with_wait_split(self, ordered):
    for _bb, insts in ordered.items():
        out = []
        for inst in insts:
            si = getattr(inst, "sync_info", None)
            if si is not None and len(si.on_wait) > _MAX_W:
                wl = list(si.on_wait)
                extra, keep = wl[:-_MAX_W], wl[-_MAX_W:]
                si.on_wait = keep
                for i in range(0, len(extra), _MAX_W):
                    nop = mybir.InstNoOp(
                        name=f"{inst.name}-wsplit{i}",
                        sync_info=mybir.SyncInfo(
                            on_wait=extra[i : i + _MAX_W], on_update=[]
                        ),
                        bass_nofuse=True,
                        engine=inst.engine,
                    )
                    out.append(nop)
            out.append(inst)
        insts[:] = out
    return _orig_lower_ordered(self, ordered)

tile.TileContext._lower_ordered_insts = _lower_with_wait_split


# ---------------------------------------------------------------------------
# host-side weight packing
# ---------------------------------------------------------------------------
def _pack_conv(w, scale=1.0):
    """[O, I, K, K] -> [n_kt, K*K, kt_size, O]  (lhsT blocks per ktile/tap)."""
    O, I, K, _ = w.shape
    kt = 128 if I >= 128 else I
    nkt = I // kt
    a = (np.asarray(w, np.float32) * scale).transpose(1, 2, 3, 0)  # [I,K,K,O]
    a = a.reshape(nkt, kt, K, K, O).transpose(0, 2, 3, 1, 4)
    return np.ascontiguousarray(a.reshape(nkt, K * K, kt, O))


def _bf(a):
    return np.asarray(a).astype(ml_dtypes.bfloat16)


def _pack_conv5(w, scale=1.0):
    """[O, I, K, K] -> [n_kt, K*K, n_mt, kt, 128]: per (ktile, tap, mtile)
    contiguous lhsT blocks for the streaming conv passes."""
    a = _pack_conv(w, scale)  # [nkt, KK, kt, O]
    nkt, kk, kt, O = a.shape
    return np.ascontiguousarray(
        a.reshape(nkt, kk, kt, O // 128, 128).transpose(0, 1, 3, 2, 4)
    )


def _prep_inputs(inp):
    """Full problem inputs -> dict of packed host arrays (shared by cores)."""
    d = {}
    # reduce conv: fold BN, duplicate output channels to fill 128 partitions
    sc = np.asarray(inp["bn_g"], np.float32) / np.sqrt(
        np.asarray(inp["bn_var"], np.float32) + EPS
    )
    w_red = np.asarray(inp["reduce_w"], np.float32)[:, :, 0, 0] * sc[:, None]  # [64,256]
    b_red = (
        np.asarray(inp["reduce_b"], np.float32) - np.asarray(inp["bn_mean"], np.float32)
    ) * sc + np.asarray(inp["bn_b"], np.float32)
    w_red2 = np.concatenate([w_red, w_red], axis=0)  # [128, 256]
    d["wred"] = _bf(_pack_conv(w_red2[:, :, None, None]))  # [2,1,128,128]
    d["bred"] = np.concatenate([b_red, b_red])[:, None].astype(np.float32)  # [128,1]

    # DWT-branch convs: input is M = 2*qkv0, so fold the 0.5 into weights
    d["w1"] = _bf(_pack_conv5(inp["conv1_w"], 0.5))
    d["w2"] = _bf(_pack_conv5(inp["conv2_w"], 0.5))
    d["w3"] = _bf(_pack_conv5(inp["conv3_w"], 0.5))
    d["w4"] = _bf(_pack_conv5(inp["conv4_w"], 0.5))
    # concat conv: groups [qkv0(=0.5*M), q1, q2, q3, q4, p]
    wcat = np.asarray(inp["conv1x1_w"], np.float32)
    packs = []
    for g in range(6):
        s = 0.5 if g == 0 else 1.0
        packs.append(_pack_conv5(wcat[:, g * 256 : (g + 1) * 256], s))
    d["wcat"] = _bf(np.concatenate(packs, axis=0))  # [12,9,2,128,128]
    d["wch"] = _bf(_pack_conv5(inp["channel_conv_w"]))  # [2,9,2,128,128]
    d["wcont32"] = _pack_conv(inp["conv_w"]).astype(np.float32)  # [2,1,128,1]
    d["wproj"] = _bf(_pack_conv(inp["proj_w"]))  # [2,1,128,256]
    d["wct1"] = _pack_conv(inp["ct1_w"]).astype(np.float32)  # [2,1,128,32]
    d["ct1b"] = np.asarray(inp["ct1_b"], np.float32)[:, None]  # [32,1]
    d["wct2"] = _pack_conv(inp["ct2_w"]).astype(np.float32)  # [1,1,32,256]
    d["ct2b"] = np.asarray(inp["ct2_b"], np.float32).reshape(2, 128, 1)
    d["lng"] = np.asarray(inp["ln_g"], np.float32)[:, None]
    d["lnb"] = np.asarray(inp["ln_b"], np.float32)[:, None]
    return d


# ---------------------------------------------------------------------------
# kernel body
# ---------------------------------------------------------------------------
def _emit(nc, tc, ap, debug=False):
    ctx = ExitStack()
    consts = ctx.enter_context(tc.tile_pool(name="consts", bufs=1))
    acts = ctx.enter_context(tc.tile_pool(name="acts", bufs=1))
    wring = ctx.enter_context(tc.tile_pool(name="wring", bufs=1))
    work = ctx.enter_context(tc.tile_pool(name="work", bufs=1))
    psum = ctx.enter_context(tc.tile_pool(name="psum", bufs=8, space="PSUM"))

    def cst(name, shape, dtype, src):
        t = consts.tile(shape, dtype, tag=name, name=name)
        nc.sync.dma_start(out=t, in_=src)
        return t

    wred = [cst(f"wred{k}", [128, 128], BF16, ap["wred"][k, 0]) for k in range(2)]
    bred = cst("bred", [128, 1], F32, ap["bred"])
    wcont32 = [cst(f"wcont32{k}", [128, 1], F32, ap["wcont32"][k, 0]) for k in range(2)]
    wproj = [cst(f"wproj{k}", [128, 256], BF16, ap["wproj"][k, 0]) for k in range(2)]
    wct1 = [cst(f"wct1{k}", [128, 32], F32, ap["wct1"][k, 0]) for k in range(2)]
    wct2 = cst("wct2", [32, 256], F32, ap["wct2"][0, 0])
    ct1b = cst("ct1b", [32, 1], F32, ap["ct1b"])
    ct2b = [cst(f"ct2b{k}", [128, 1], F32, ap["ct2b"][k]) for k in range(2)]
    lng = cst("lng", [32, 1], F32, ap["lng"])
    lnb = cst("lnb", [32, 1], F32, ap["lnb"])

    sigma = consts.tile([128, 1], F32, tag="sigma", name="sigma")
    nc.vector.memset(sigma[0:64, :], 1.0)
    nc.vector.memset(sigma[64:128, :], -1.0)
    epsv = consts.tile([32, 1], F32, tag="epsv", name="epsv")
    nc.vector.memset(epsv, EPS)
    onesb = consts.tile([1, 128], BF16, tag="onesb", name="onesb")
    nc.vector.memset(onesb, 1.0)
    onesf = consts.tile([32, 1], F32, tag="onesf", name="onesf")
    nc.vector.memset(onesf, 1.0)
    onesf2 = consts.tile([1, 32], F32, tag="onesf2", name="onesf2")
    nc.vector.memset(onesf2, 1.0)

    # padded activation buffers (bf16), reused across images via same tags
    def padbuf(name, hw):
        return acts.tile([128, hw, hw], BF16, tag=name, name=name)

    for img in range(BPC):
        q0 = [padbuf(f"q0_{k}", 70) for k in range(2)]  # M, origin (3,3)
        pb = [padbuf(f"p_{k}", 66) for k in range(2)]  # maxpool, origin (1,1)
        qb = [[padbuf(f"q{j}_{k}", 66) for k in range(2)] for j in range(1, 5)]
        qkv = [padbuf(f"qkv_{k}", 66) for k in range(2)]

        # zero the halo borders (interior is fully overwritten)
        for t in [*q0]:
            nc.gpsimd.memset(t[:, 0:3, :], 0.0)
            nc.gpsimd.memset(t[:, 67:70, :], 0.0)
            nc.gpsimd.memset(t[:, 3:67, 0:3], 0.0)
            nc.gpsimd.memset(t[:, 3:67, 67:70], 0.0)
        for t in [*pb, *qb[0], *qb[1], *qb[2], *qb[3], *qkv]:
            nc.gpsimd.memset(t[:, 0:1, :], 0.0)
            nc.gpsimd.memset(t[:, 65:66, :], 0.0)
            nc.gpsimd.memset(t[:, 1:65, 0:1], 0.0)
            nc.gpsimd.memset(t[:, 1:65, 65:66], 0.0)

        # ---- phase 1: stream x, reduce conv + ReLU -> DWT -> M;  maxpool -> p
        for sc_ in range(16):  # 8 input rows per superchunk
            xts = []
            for k in range(2):
                xt = work.tile([128, 8, 128], BF16, tag=f"x{k}", bufs=3, name=f"xt{k}")
                nc.sync.dma_start(
                    out=xt, in_=ap["x"][img, k * 128 : (k + 1) * 128, sc_ * 8 : sc_ * 8 + 8, :]
                )
                xts.append(xt)
            orow = sc_ * 4  # 8 input rows -> 4 output rows per superchunk
            rch = work.tile([128, 8, 128], F32, tag="rch", bufs=2, name="rch")
            for sub in range(2):
                ps = psum.tile([128, 4, 128], F32, tag="ps", name="ps_r")
                for k in range(2):
                    nc.tensor.matmul(
                        ps, wred[k], xts[k][:, sub * 4 : sub * 4 + 4, :],
                        start=(k == 0), stop=(k == 1),
                    )
                nc.scalar.activation(
                    out=rch[:, sub * 4 : sub * 4 + 4, :], in_=ps, func=AF.Relu,
                    bias=bred, scale=1.0,
                )
            rv = rch.rearrange("p (a two) (c cp) -> p a two c cp", two=2, cp=2)
            a_, b_ = rv[:, :, 0, :, 0], rv[:, :, 0, :, 1]
            c_, d_ = rv[:, :, 1, :, 0], rv[:, :, 1, :, 1]
            u = work.tile([128, 4, 64], F32, tag="u", bufs=2, name="u")
            v = work.tile([128, 4, 64], F32, tag="v", bufs=2, name="v")
            s_ = work.tile([128, 4, 64], F32, tag="s", bufs=2, name="s_")
            t_ = work.tile([128, 4, 64], F32, tag="t", bufs=2, name="t_")
            nc.vector.tensor_add(u, a_, b_)
            nc.vector.tensor_add(v, c_, d_)
            nc.vector.tensor_sub(s_, a_, b_)
            nc.vector.tensor_sub(t_, c_, d_)
            sv = work.tile([128, 4, 64], F32, tag="sv", bufs=2, name="sv")
            st = work.tile([128, 4, 64], F32, tag="st", bufs=2, name="st")
            # sigma-scale on the Scalar engine to unload DVE
            nc.scalar.activation(out=sv, in_=v, func=AF.Copy, scale=sigma)
            nc.scalar.activation(out=st, in_=t_, func=AF.Copy, scale=sigma)
            nc.vector.tensor_add(q0[0][:, 3 + orow : 7 + orow, 3:67], u, sv)
            nc.vector.tensor_add(q0[1][:, 3 + orow : 7 + orow, 3:67], s_, st)
            for k in range(2):
                xv = xts[k].rearrange("p (a two) (c cp) -> p a two c cp", two=2, cp=2)
                xa = xv[:, :, 0, :, 0]
                xb = xv[:, :, 0, :, 1]
                xc = xv[:, :, 1, :, 0]
                xd = xv[:, :, 1, :, 1]
                m1 = work.tile([128, 4, 64], BF16, tag="m1", bufs=2, name="m1")
                m2 = work.tile([128, 4, 64], BF16, tag="m2", bufs=2, name="m2")
                nc.vector.tensor_max(m1, xa, xb)
                nc.vector.tensor_max(m2, xc, xd)
                nc.vector.tensor_max(pb[k][:, 1 + orow : 5 + orow, 1:65], m1, m2)

        # ---- phase 2: the four DWT-branch convs + concat conv
        def conv_pass(wdram, n_k, K, rhs_fn, out_fn, wtag):
            """accumulate over (ktile, tap) into 8 psum banks (2 mt x 4 chunks)"""
            for qh in range(2):
                pss = [
                    [
                        psum.tile([128, 8, 64], F32, tag="ps", name="ps_c")
                        for _ in range(4)
                    ]
                    for _ in range(2)
                ]
                for ik in range(n_k):
                    for tp in range(K * K):
                        wt = wring.tile(
                            [128, 256], BF16, tag=wtag, bufs=10, name="wt"
                        )
                        nc.sync.dma_start(out=wt, in_=wdram[ik, tp].rearrange("m p c -> p m c"))
                        for mt in range(2):
                            lhsT = wt[:, mt * 128 : (mt + 1) * 128]
                            for ci in range(4):
                                r0 = qh * 32 + ci * 8
                                nc.tensor.matmul(
                                    pss[mt][ci], lhsT, rhs_fn(ik, tp, r0),
                                    start=(ik == 0 and tp == 0),
                                    stop=(ik == n_k - 1 and tp == K * K - 1),
                                )
                for ci in range(4):
                    for mt in range(2):
                        out_fn(mt, qh * 32 + ci * 8, pss[mt][ci])

        for j, K in ((0, 1), (1, 3), (2, 5), (3, 7)):
            base = 3 - (K // 2)
            dst = qb[j]

            def rhs_m(ik, tp, r0, K=K, base=base):
                ky, kx = tp // K, tp % K
                return q0[ik][:, base + ky + r0 : base + ky + r0 + 8, base + kx : base + kx + 64]

            def wr(mt, r0, ps_, dst=dst):
                nc.vector.tensor_copy(dst[mt][:, 1 + r0 : 9 + r0, 1:65], ps_)

            conv_pass(ap[f"w{j+1}"], 2, K, rhs_m, wr, "wtap")

        def rhs_cat(ik, tp, r0):
            g, k = ik // 2, ik % 2
            ky, kx = tp // 3, tp % 3
            if g == 0:
                return q0[k][:, 2 + ky + r0 : 2 + ky + r0 + 8, 2 + kx : 2 + kx + 64]
            src = pb[k] if g == 5 else qb[g - 1][k]
            return src[:, ky + r0 : ky + r0 + 8, kx : kx + 64]

        # concat conv drain also stages fp32 chunks and runs the content conv
        # on them (fp32), accumulating logits into content_sb on partition 0.
        content_sb = work.tile([1, 64, 64], F32, tag="content", name="content_sb")
        qs32 = {}

        def wr_cat(mt, r0, ps_):
            nc.vector.tensor_copy(qkv[mt][:, 1 + r0 : 9 + r0, 1:65], ps_)
            st = work.tile([128, 8, 64], F32, tag="st32", bufs=3, name="st")
            nc.scalar.copy(st, ps_)
            qs32[mt] = st
            if mt == 1:
                cp = psum.tile([1, 8, 64], F32, tag="ps", name="cp")
                nc.tensor.matmul(cp, wcont32[0], qs32[0], start=True, stop=False)
                nc.tensor.matmul(cp, wcont32[1], qs32[1], start=False, stop=True)
                nc.vector.tensor_copy(content_sb[:, r0 : r0 + 8, :], cp)

        conv_pass(ap["wcat"], 12, 3, rhs_cat, wr_cat, "wtap")

        # ---- phase 3: softmax on partition 0; 1/denominator folded into the
        # broadcast of e across partitions.  No max-subtraction: the logits
        # for this problem's input distribution stay well inside fp32 exp
        # range (|content| < ~35 << 88), and the e/den ratio is unchanged.
        e_bf = work.tile([1, 64, 64], BF16, tag="ebf", name="e_bf")
        den = work.tile([1, 1], F32, tag="den", name="den")
        nc.scalar.activation(
            out=e_bf, in_=content_sb, func=AF.Exp, bias=0.0, scale=1.0,
            accum_out=den,
        )
        rden = work.tile([1, 1], F32, tag="rden", name="rden")
        nc.vector.reciprocal(rden, den)
        ones_sc = work.tile([1, 128], BF16, tag="ones_sc", name="ones_sc")
        nc.vector.tensor_scalar_mul(ones_sc, onesb, rden)
        # ebc[p, n] = e[n] / den  for all partitions p
        ebc = work.tile([128, 64, 64], BF16, tag="ebc", name="ebc")
        for ci in range(8):
            eb_ps = psum.tile([128, 8, 64], F32, tag="ps", name="eb_ps")
            nc.tensor.matmul(
                eb_ps, ones_sc, e_bf[:, ci * 8 : (ci + 1) * 8, :],
                start=True, stop=True,
            )
            nc.scalar.copy(ebc[:, ci * 8 : (ci + 1) * 8, :], eb_ps)

        # channel conv (standard orientation) fused with attention pooling:
        # pooled[c] = sum_n channel[c, n] * ebc[c, n]
        partials = [
            work.tile([128, 8], F32, tag=f"part{mt}", name="partials") for mt in range(2)
        ]

        def wr_ch(mt, r0, ps_):
            ttr = work.tile([128, 8, 64], F32, tag="st32", bufs=3, name="ttr")
            nc.vector.tensor_mul(ttr, ps_, ebc[:, r0 : r0 + 8, :])
            nc.vector.tensor_reduce(
                partials[mt][:, r0 // 8 : r0 // 8 + 1], ttr,
                axis=mybir.AxisListType.XY, op=mybir.AluOpType.add,
            )

        def rhs_ch(ik, tp, r0):
            ky, kx = tp // 3, tp % 3
            return qkv[ik][:, ky + r0 : ky + r0 + 8, kx : kx + 64]

        conv_pass(ap["wch"], 2, 3, rhs_ch, wr_ch, "wtap")
        pooled = []
        for mt in range(2):
            pl = work.tile([128, 1], F32, tag=f"pool{mt}", name="pl")
            nc.vector.tensor_reduce(
                pl, partials[mt], axis=mybir.AxisListType.X, op=mybir.AluOpType.add
            )
            pooled.append(pl)

        # ---- phase 4: channel transform (tiny, fp32)
        t_ps = psum.tile([32, 1], F32, tag="ps", name="t_ps")
        for k in range(2):
            nc.tensor.matmul(t_ps, wct1[k], pooled[k], start=(k == 0), stop=(k == 1))
        ts2 = work.tile([32, 2], F32, tag="ts2", name="ts2")
        t_sb = ts2[:, 0:1]
        nc.vector.tensor_scalar_add(t_sb, t_ps, ct1b)
        nc.vector.tensor_mul(ts2[:, 1:2], t_sb, t_sb)
        # cross-partition sums of (t, t^2) via fp32 ones-matmul, broadcast back
        sums_ps = psum.tile([1, 2], F32, tag="ps", name="sums_ps")
        nc.tensor.matmul(sums_ps, onesf, ts2, start=True, stop=True)
        sums_sb = work.tile([1, 2], F32, tag="sums_sb", name="sums_sb")
        nc.vector.tensor_copy(sums_sb, sums_ps)
        bc_ps = psum.tile([32, 2], F32, tag="ps", name="bc_ps")
        nc.tensor.matmul(bc_ps, onesf2, sums_sb, start=True, stop=True)
        mean = work.tile([32, 1], F32, tag="mean", name="mean")
        nc.vector.tensor_scalar_mul(mean, bc_ps[:, 0:1], 1.0 / DS)
        mv = work.tile([32, 1], F32, tag="mv", name="mv")
        nc.vector.tensor_scalar_mul(mv, bc_ps[:, 1:2], 1.0 / DS)
        m2t = work.tile([32, 1], F32, tag="m2t", name="m2t")
        nc.vector.tensor_mul(m2t, mean, mean)
        var = work.tile([32, 1], F32, tag="var", name="var")
        nc.vector.tensor_sub(var, mv, m2t)
        sd = work.tile([32, 1], F32, tag="sd", name="sd")
        nc.scalar.activation(out=sd, in_=var, func=AF.Sqrt, bias=epsv, scale=1.0)
        rsd = work.tile([32, 1], F32, tag="rsd", name="rsd")
        nc.vector.reciprocal(rsd, sd)
        dt_ = work.tile([32, 1], F32, tag="dt", name="dt_")
        nc.vector.tensor_sub(dt_, t_sb, mean)
        tn = work.tile([32, 1], F32, tag="tn", name="tn")
        nc.vector.tensor_mul(tn, dt_, rsd)
        tact = work.tile([32, 1], F32, tag="tact", name="tact")
        nc.scalar.activation(out=tact, in_=tn, func=AF.Relu, bias=lnb, scale=lng)

        projs = []
        for mt in range(2):
            cw_ps = psum.tile([128, 1], F32, tag="ps", name="cw_ps")
            nc.tensor.matmul(cw_ps, wct2[:, mt * 128 : (mt + 1) * 128], tact, start=True, stop=True)
            cw = work.tile([128, 1], F32, tag=f"cw{mt}", name="cw")
            nc.vector.tensor_scalar_add(cw, cw_ps, ct2b[mt])
            pj = work.tile([128, 256], BF16, tag=f"projs{mt}", name="pj")
            nc.vector.tensor_scalar_mul(pj, wproj[mt], cw)
            projs.append(pj)

        if debug:
            for k in range(2):
                nc.sync.dma_start(out=ap["dbg_m"][img, k], in_=q0[k])
                nc.sync.dma_start(out=ap["dbg_p"][img, k], in_=pb[k])
                nc.sync.dma_start(out=ap["dbg_qkv"][img, k], in_=qkv[k])
                nc.sync.dma_start(out=ap["dbg_cw"][img, k], in_=projs[k])
                nc.sync.dma_start(out=ap["dbg_pool"][img, k], in_=pooled[k])
            nc.sync.dma_start(out=ap["dbg_e"][img], in_=ebc[0:1])

        # ---- phase 5: out = proj(qkv * cw)  (cw folded into proj weights)
        for mt in range(2):
            for ci in range(8):
                r0 = ci * 8
                po = psum.tile([128, 8, 64], F32, tag="ps", name="po")
                for k in range(2):
                    nc.tensor.matmul(
                        po,
                        projs[k][:, mt * 128 : (mt + 1) * 128],
                        qkv[k][:, 1 + r0 : 9 + r0, 1:65],
                        start=(k == 0), stop=(k == 1),
                    )
                ost = work.tile([128, 8, 64], F32, tag="st32", bufs=3, name="ost")
                nc.scalar.copy(ost, po)
                nc.sync.dma_start(
                    out=ap["out"][img, mt * 128 : (mt + 1) * 128, r0 : r0 + 8, :],
                    in_=ost,
                )
    ctx.close()


def build(debug=False):
    nc = bass.Bass("TRN2", target_bir_lowering=False, debug=False)
    shapes = {
        "x": ([BPC, C, H, W], BF16),
        "wred": ([2, 1, 128, 128], BF16),
        "bred": ([128, 1], F32),
        "w1": ([2, 1, 2, 128, 128], BF16),
        "w2": ([2, 9, 2, 128, 128], BF16),
        "w3": ([2, 25, 2, 128, 128], BF16),
        "w4": ([2, 49, 2, 128, 128], BF16),
        "wcat": ([12, 9, 2, 128, 128], BF16),
        "wch": ([2, 9, 2, 128, 128], BF16),
        "wcont32": ([2, 1, 128, 1], F32),
        "wproj": ([2, 1, 128, 256], BF16),
        "wct1": ([2, 1, 128, 32], F32),
        "ct1b": ([32, 1], F32),
        "wct2": ([1, 1, 32, 256], F32),
        "ct2b": ([2, 128, 1], F32),
        "lng": ([32, 1], F32),
        "lnb": ([32, 1], F32),
    }
    ap = {
        k: nc.dram_tensor(k, shp, dt, kind="ExternalInput").ap()
        for k, (shp, dt) in shapes.items()
    }
    ap["out"] = nc.dram_tensor("out", [BPC, C, H2, W2], F32, kind="ExternalOutput").ap()
    if debug:
        dbg = {
            "dbg_m": ([BPC, 2, 128, 70, 70], BF16),
            "dbg_p": ([BPC, 2, 128, 66, 66], BF16),
            "dbg_qkv": ([BPC, 2, 128, 66, 66], BF16),
            "dbg_cw": ([BPC, 2, 128, 256], BF16),
            "dbg_pool": ([BPC, 2, 128, 1], F32),
            "dbg_e": ([BPC, 1, 64, 64], BF16),
        }
        for k, (shp, dt) in dbg.items():
            ap[k] = nc.dram_tensor(k, shp, dt, kind="ExternalOutput").ap()
    with tile.TileContext(nc) as tc:
        _emit(nc, tc, ap, debug=debug)
    return nc


_CACHED_NC = {}


def _install_trace_hook():
    """The image's antenv lacks axon_hooks; shim it and register the boot's
    ctypes NTFF hook so trace=True works.  Also neutralize the S3 artifact
    upload (no bucket access here)."""
    import types
    import antenv

    if "antenv.axon_hooks" not in sys.modules:
        mod = types.ModuleType("antenv.axon_hooks")
        mod._hook = None
        def set_axon_ntff_profile_hook(h):
            mod._hook = h
        def get_axon_ntff_profile_hook():
            return mod._hook
        mod.set_axon_ntff_profile_hook = set_axon_ntff_profile_hook
        mod.get_axon_ntff_profile_hook = get_axon_ntff_profile_hook
        sys.modules["antenv.axon_hooks"] = mod
        antenv.axon_hooks = mod
        from trn_agent_boot.trn_boot import _ntff_profile_via_ctypes
        mod.set_axon_ntff_profile_hook(
            _ntff_profile_via_ctypes("/opt/axon/libaxon_pjrt.so")
        )
        bass_utils.upload_artifacts = lambda tmpdir: tmpdir


def run(inputs, debug=False, trace=False):
    if trace:
        _install_trace_hook()
    key = (debug,)
    if key not in _CACHED_NC:
        _CACHED_NC[key] = build(debug=debug)
    nc = _CACHED_NC[key]
    d = _prep_inputs(inputs)
    x_bf = _bf(np.asarray(inputs["x"], np.float32))
    in_maps = []
    for c in range(N_CORES):
        m = dict(d)
        m["x"] = np.ascontiguousarray(x_bf[c * BPC : (c + 1) * BPC])
        in_maps.append(m)
    res = bass_utils.run_bass_kernel_spmd(
        nc, in_maps, core_ids=list(range(N_CORES)), trace=trace
    )
    out = np.concatenate([res.results[c]["out"] for c in range(N_CORES)], axis=0)
    return out, res


def kernel(**inputs):
    out, _ = run(inputs)
    return out



# revision 9
# speedup vs baseline: 1.0501x; 1.0501x over previous
"""Trainium2 Bass kernel for nn_Dwtpool (dense_cnn).

Reference graph (per image, C=256, 128x128 input):
  p    = maxpool2x2(x)                          -> [256, 64, 64]
  r    = ReLU(BN(conv1x1(x, reduce_w)))         -> [ 64,128,128]
  M    = haar_dwt(r) * 2  (stored unscaled)     -> [256, 64, 64]
  q2..q4 = conv{3,5,7}(0.5*M)                   -> [256, 64, 64] each
  qkv  = conv3x3(concat[0.5*M, q1..q4, p])      -> [256, 64, 64]
  att  = softmax_spatial(conv1x1(qkv)); pooled = sum_n ch(qkv)_c,n * att_n
  cw   = ct2(ReLU(LN(ct1(pooled))))             -> [256]
  out  = conv1x1(qkv * cw, proj_w)              -> [256, 64, 64]

Strategy: data-parallel over batch (16 images / 8 cores = 2 per core), fp16
trunk (same PE speed as bf16, 8x less noise; the softmax path amplifies qkv
noise ~3x into the output so 16-bit stays mandatory).  All convs are
tap-accumulated f16 matmuls into PSUM.  Algebraic cuts vs the naive graph:
  * q1 (1x1 conv) is folded into concat-conv group 0 on the host
    (conv3x3(W1, conv1x1(c1, x)) == conv3x3(W1 . c1, x)), removing one conv
    and one concat group.
  * the channel conv never materializes: pooled = sum_n a_n*conv(qkv)[:,n]
    == wch . s where s[i,tap] = sum_n a_n * qkv[i, n+tap], computed on the
    DVE with fused tensor_tensor_reduce, then an 18-matmul matvec.
  * content logits are a f16 matmul on the qkv tiles (replicated onto 8
    partitions so exp/accum runs 8-wide), softmax denominator folded into
    the e-broadcast ones-vector.
Phase-1 (reduce+DWT+maxpool) of image 1 is interleaved at the conv-pass PSUM
boundaries of image 0, and image 0's attention tail + proj run inside image
1's first conv pass, keeping the PE stream dense.
"""
import os
import sys

for _p in ("/opt/trn_rl_repo", os.path.expanduser("~/.axon_site/_ro/trn_rl_repo")):
    if os.path.isdir(_p) and _p not in sys.path:
        sys.path.append(_p)

import numpy as np
import ml_dtypes
from contextlib import ExitStack

import concourse.bass as bass
import concourse.tile as tile
from concourse import mybir
from concourse import bass_utils

BF16 = mybir.dt.bfloat16
F16 = mybir.dt.float16
F32 = mybir.dt.float32
AF = mybir.ActivationFunctionType
ALU = mybir.AluOpType

B, C, H, W = 16, 256, 128, 128
H2, W2 = 64, 64
N_CORES = 8
BPC = B // N_CORES  # images per core
EPS = 1e-5

# ---------------------------------------------------------------------------
# walrus CoreV3 rejects instructions with more than a couple of sync waits;
# Tile's exit drain accumulates one wait per processor used.  Split the waits
# across a chain of drain instructions (sync engine executes them in order).
# ---------------------------------------------------------------------------
import bass_rust as _br
import concourse.tile as _tile_mod

def _split_drain_and_barrier(self, tick_clock, wait_clock):
    nc = self.nc
    drain_inst = nc.sync.drain()
    wait_clock.add_sem_waits(
        drain_inst.ins, _tile_mod.ScopedClock({None: tick_clock.global_clock})
    )
    W_ = list(drain_inst.ins.sync_info.on_wait)
    if len(W_) > 1:
        drain_inst.ins.sync_info.on_wait = W_[:1]
        for i in range(1, len(W_)):
            extra = nc.sync.drain()
            extra.ins.sync_info = _br.SyncInfo(on_wait=W_[i : i + 1], on_update=[])
    nc.all_engine_barrier()
    assert self.sems is not None
    popped = nc._tile_sem_poison_stack.pop()
    assert popped is self._sem_poison
    nc.clear_and_free_semaphores(list(self.sems.allocated().values()))
    nc.all_engine_barrier()

tile.TileContext._drain_and_barrier = _split_drain_and_barrier

# Same hardware limit applies to scheduled body instructions (max 2 sync waits
# per instruction).  Before lowering, move excess waits onto injected NOPs on
# the same engine.
_MAX_W = 1
_orig_lower_ordered = tile.TileContext._lower_ordered_insts

def _lower_with_wait_split(self, ordered):
    for _bb, insts in ordered.items():
        out = []
        for inst in insts:
            si = getattr(inst, "sync_info", None)
            if si is not None and len(si.on_wait) > _MAX_W:
                wl = list(si.on_wait)
                extra, keep = wl[:-_MAX_W], wl[-_MAX_W:]
                si.on_wait = keep
                for i in range(0, len(extra), _MAX_W):
                    nop = mybir.InstNoOp(
                        name=f"{inst.name}-wsplit{i}",
                        sync_info=mybir.SyncInfo(
                            on_wait=extra[i : i + _MAX_W], on_update=[]
                        ),
                        bass_nofuse=True,
                        engine=inst.engine,
                    )
                    out.append(nop)
            out.append(inst)
        insts[:] = out
    return _orig_lower_ordered(self, ordered)

tile.TileContext._lower_ordered_insts = _lower_with_wait_split


# ---------------------------------------------------------------------------
# host-side weight packing
# ---------------------------------------------------------------------------
def _pack_conv(w, scale=1.0):
    """[O, I, K, K] -> [n_kt, K*K, kt_size, O]  (lhsT blocks per ktile/tap)."""
    O, I, K, _ = w.shape
    kt = 128 if I >= 128 else I
    nkt = I // kt
    a = (np.asarray(w, np.float32) * scale).transpose(1, 2, 3, 0)  # [I,K,K,O]
    a = a.reshape(nkt, kt, K, K, O).transpose(0, 2, 3, 1, 4)
    return np.ascontiguousarray(a.reshape(nkt, K * K, kt, O))


def _f16(a):
    return np.asarray(a).astype(np.float16)


def _pack_conv5(w, scale=1.0):
    """[O, I, K, K] -> [n_kt, K*K, n_mt, kt, 128]: per (ktile, tap, mtile)
    contiguous lhsT blocks for the streaming conv passes."""
    a = _pack_conv(w, scale)  # [nkt, KK, kt, O]
    nkt, kk, kt, O = a.shape
    return np.ascontiguousarray(
        a.reshape(nkt, kk, kt, O // 128, 128).transpose(0, 1, 3, 2, 4)
    )


def _prep_inputs(inp):
    """Full problem inputs -> dict of packed host arrays (shared by cores)."""
    d = {}
    # reduce conv: fold BN, duplicate output channels to fill 128 partitions
    sc = np.asarray(inp["bn_g"], np.float32) / np.sqrt(
        np.asarray(inp["bn_var"], np.float32) + EPS
    )
    w_red = np.asarray(inp["reduce_w"], np.float32)[:, :, 0, 0] * sc[:, None]  # [64,256]
    b_red = (
        np.asarray(inp["reduce_b"], np.float32) - np.asarray(inp["bn_mean"], np.float32)
    ) * sc + np.asarray(inp["bn_b"], np.float32)
    w_red2 = np.concatenate([w_red, w_red], axis=0)  # [128, 256]
    d["wred"] = _f16(_pack_conv(w_red2[:, :, None, None]))  # [2,1,128,128]
    d["bred"] = np.concatenate([b_red, b_red])[:, None].astype(np.float32)  # [128,1]

    # DWT-branch convs: input is M = 2*qkv0, so fold the 0.5 into weights
    d["w2"] = _f16(_pack_conv5(inp["conv2_w"], 0.5))
    d["w3"] = _f16(_pack_conv5(inp["conv3_w"], 0.5))
    d["w4"] = _f16(_pack_conv5(inp["conv4_w"], 0.5))
    # concat conv groups [qkv0(=0.5*M), q2, q3, q4, p]; the q1 group is folded
    # into group 0: conv3x3(W1, conv1x1(c1, qkv0)) == conv3x3(W1 . c1, qkv0)
    wcat = np.asarray(inp["conv1x1_w"], np.float32)
    conv1 = np.asarray(inp["conv1_w"], np.float32)[:, :, 0, 0]  # [256,256]
    g0 = wcat[:, 0:256] + np.einsum("ocyx,ci->oiyx", wcat[:, 256:512], conv1)
    packs = [_pack_conv5(g0, 0.5)]
    for g in range(2, 6):
        packs.append(_pack_conv5(wcat[:, g * 256 : (g + 1) * 256]))
    d["wcat"] = _f16(np.concatenate(packs, axis=0))  # [10,9,2,128,128]
    d["wch"] = _f16(_pack_conv5(inp["channel_conv_w"]))  # [2,9,2,128,128]
    wc = np.asarray(inp["conv_w"], np.float32)[0, :, 0, 0]  # [256]
    d["wcont16"] = _f16(wc.reshape(2, 128, 1))  # [2,128,1]
    d["wproj"] = _f16(_pack_conv(inp["proj_w"]))  # [2,1,128,256]
    d["wct1"] = _pack_conv(inp["ct1_w"]).astype(np.float32)  # [2,1,128,32]
    d["ct1b"] = np.asarray(inp["ct1_b"], np.float32)[:, None]  # [32,1]
    d["wct2"] = _pack_conv(inp["ct2_w"]).astype(np.float32)  # [1,1,32,256]
    d["ct2b"] = np.asarray(inp["ct2_b"], np.float32).reshape(2, 128, 1)
    d["lng"] = np.asarray(inp["ln_g"], np.float32)[:, None]
    d["lnb"] = np.asarray(inp["ln_b"], np.float32)[:, None]
    return d


# ---------------------------------------------------------------------------
# kernel body
# ---------------------------------------------------------------------------
def _emit(nc, tc, ap):
    ctx = ExitStack()
    consts = ctx.enter_context(tc.tile_pool(name="consts", bufs=1))
    acts = ctx.enter_context(tc.tile_pool(name="acts", bufs=1))
    wring = ctx.enter_context(tc.tile_pool(name="wring", bufs=1))
    work = ctx.enter_context(tc.tile_pool(name="work", bufs=1))
    psum = ctx.enter_context(tc.tile_pool(name="psum", bufs=8, space="PSUM"))

    def cst(name, shape, dtype, src):
        t = consts.tile(shape, dtype, tag=name, name=name)
        nc.sync.dma_start(out=t, in_=src)
        return t

    wred = [cst(f"wred{k}", [128, 128], F16, ap["wred"][k, 0]) for k in range(2)]
    bred = cst("bred", [128, 1], F32, ap["bred"])
    wcont16 = [cst(f"wcont16{k}", [128, 1], F16, ap["wcont16"][k]) for k in range(2)]
    wproj = [cst(f"wproj{k}", [128, 256], F16, ap["wproj"][k, 0]) for k in range(2)]
    wct1 = [cst(f"wct1{k}", [128, 32], F32, ap["wct1"][k, 0]) for k in range(2)]
    wct2 = cst("wct2", [32, 256], F32, ap["wct2"][0, 0])
    ct1b = cst("ct1b", [32, 1], F32, ap["ct1b"])
    ct2b = [cst(f"ct2b{k}", [128, 1], F32, ap["ct2b"][k]) for k in range(2)]
    lng = cst("lng", [32, 1], F32, ap["lng"])
    lnb = cst("lnb", [32, 1], F32, ap["lnb"])

    sigma = consts.tile([128, 1], F32, tag="sigma", name="sigma")
    nc.vector.memset(sigma[0:64, :], 1.0)
    nc.vector.memset(sigma[64:128, :], -1.0)
    epsv = consts.tile([32, 1], F32, tag="epsv", name="epsv")
    nc.vector.memset(epsv, EPS)
    onesb = consts.tile([1, 128], BF16, tag="onesb", name="onesb")
    nc.vector.memset(onesb, 1.0)
    onesf = consts.tile([32, 1], F32, tag="onesf", name="onesf")
    nc.vector.memset(onesf, 1.0)
    onesf2 = consts.tile([1, 32], F32, tag="onesf2", name="onesf2")
    nc.vector.memset(onesf2, 1.0)

    # ---- activation buffers (f16).  q0/pb are per-image (phase-1 of image
    # i+1 overlaps image i's conv passes); qb/qkv are shared (WAR deps order
    # them behind the previous image's reads, which is late enough).
    def padbuf(name, hw):
        return acts.tile([128, hw, hw], F16, tag=name, name=name)

    q0 = [[padbuf(f"q0_{im}_{k}", 70) for k in range(2)] for im in range(BPC)]
    pb = [[padbuf(f"p_{im}_{k}", 66) for k in range(2)] for im in range(BPC)]
    qb = [[padbuf(f"q{j}_{k}", 66) for k in range(2)] for j in range(3)]
    qkv = [padbuf(f"qkv_{k}", 66) for k in range(2)]

    # zero the halo borders once (interior is fully overwritten per image)
    for t in [t_ for im in range(BPC) for t_ in q0[im]]:
        nc.gpsimd.memset(t[:, 0:3, :], 0.0)
        nc.gpsimd.memset(t[:, 67:70, :], 0.0)
        nc.gpsimd.memset(t[:, 3:67, 0:3], 0.0)
        nc.gpsimd.memset(t[:, 3:67, 67:70], 0.0)
    for t in [t_ for im in range(BPC) for t_ in pb[im]] + [
        t_ for j in range(3) for t_ in qb[j]
    ] + qkv:
        nc.gpsimd.memset(t[:, 0:1, :], 0.0)
        nc.gpsimd.memset(t[:, 65:66, :], 0.0)
        nc.gpsimd.memset(t[:, 1:65, 0:1], 0.0)
        nc.gpsimd.memset(t[:, 1:65, 65:66], 0.0)

    # ---- phase 1: stream x -> reduce conv+ReLU -> DWT -> M;  maxpool -> p
    class Ph1:
        def __init__(self, img):
            self.img = img
            self.xts = []
            self.sc = 0

        def emit_dmas(self):
            for sc_ in range(16):
                pair = []
                for k in range(2):
                    xt = work.tile(
                        [128, 8, 128], F16, tag=f"x{k}", bufs=3, name=f"xt{k}"
                    )
                    src = ap["x"][
                        self.img, k * 128 : (k + 1) * 128, sc_ * 8 : sc_ * 8 + 8, :
                    ]
                    nc.sync.dma_start(out=xt[:, 0:4, :], in_=src[:, 0:4, :])
                    nc.sync.dma_start(out=xt[:, 4:8, :], in_=src[:, 4:8, :])
                    pair.append(xt)
                self.xts.append(pair)

        def step(self, n=1):
            for _ in range(n):
                if self.sc < 16:
                    self._sc(self.sc)
                    self.sc += 1

        def _sc(self, sc_):
            xts = self.xts[sc_]
            orow = sc_ * 4  # 8 input rows -> 4 output rows
            rch = work.tile([128, 8, 128], F16, tag="rch", bufs=2, name="rch")
            for sub in range(2):
                ps = psum.tile([128, 4, 128], F32, tag="ps", name="ps_r")
                for k in range(2):
                    nc.tensor.matmul(
                        ps, wred[k], xts[k][:, sub * 4 : sub * 4 + 4, :],
                        start=(k == 0), stop=(k == 1),
                    )
                nc.scalar.activation(
                    out=rch[:, sub * 4 : sub * 4 + 4, :], in_=ps, func=AF.Relu,
                    bias=bred, scale=1.0,
                )
            rv = rch.rearrange("p (a two) (c cp) -> p a two c cp", two=2, cp=2)
            a_, b_ = rv[:, :, 0, :, 0], rv[:, :, 0, :, 1]
            c_, d_ = rv[:, :, 1, :, 0], rv[:, :, 1, :, 1]
            u = work.tile([128, 4, 64], F32, tag="u", bufs=2, name="u")
            v = work.tile([128, 4, 64], F32, tag="v", bufs=2, name="v")
            s_ = work.tile([128, 4, 64], F32, tag="s", bufs=2, name="s_")
            t_ = work.tile([128, 4, 64], F32, tag="t", bufs=2, name="t_")
            nc.vector.tensor_add(u, a_, b_)
            nc.vector.tensor_add(v, c_, d_)
            nc.vector.tensor_sub(s_, a_, b_)
            nc.vector.tensor_sub(t_, c_, d_)
            sv = work.tile([128, 4, 64], F32, tag="sv", bufs=2, name="sv")
            st = work.tile([128, 4, 64], F32, tag="st", bufs=2, name="st")
            # sigma-scale on the Scalar engine to unload DVE
            nc.scalar.activation(out=sv, in_=v, func=AF.Copy, scale=sigma)
            nc.scalar.activation(out=st, in_=t_, func=AF.Copy, scale=sigma)
            myq0 = q0[self.img]
            nc.vector.tensor_add(myq0[0][:, 3 + orow : 7 + orow, 3:67], u, sv)
            nc.vector.tensor_add(myq0[1][:, 3 + orow : 7 + orow, 3:67], s_, st)
            for k in range(2):
                xv = xts[k].rearrange("p (a two) (c cp) -> p a two c cp", two=2, cp=2)
                xa, xb = xv[:, :, 0, :, 0], xv[:, :, 0, :, 1]
                xc, xd = xv[:, :, 1, :, 0], xv[:, :, 1, :, 1]
                m1 = work.tile([128, 4, 64], F16, tag="m1", bufs=2, name="m1")
                m2 = work.tile([128, 4, 64], F16, tag="m2", bufs=2, name="m2")
                nc.vector.tensor_max(m1, xa, xb)
                nc.vector.tensor_max(m2, xc, xd)
                nc.vector.tensor_max(pb[self.img][k][:, 1 + orow : 5 + orow, 1:65], m1, m2)

    # ---- conv pass: accumulate over (ktile, tap) into 8 psum banks
    def conv_pass(wdram, n_k, K, rhs_fn, out_fn, boundary_hook=None):
        for qh in range(2):
            pss = [
                [psum.tile([128, 8, 64], F32, tag="ps", name="ps_c") for _ in range(4)]
                for _ in range(2)
            ]
            for ik in range(n_k):
                for tp in range(K * K):
                    wt = wring.tile([128, 256], F16, tag="wtap", bufs=8, name="wt")
                    nc.sync.dma_start(
                        out=wt, in_=wdram[ik, tp].rearrange("m p c -> p m c")
                    )
                    for mt in range(2):
                        lhsT = wt[:, mt * 128 : (mt + 1) * 128]
                        for ci in range(4):
                            r0 = qh * 32 + ci * 8
                            nc.tensor.matmul(
                                pss[mt][ci], lhsT, rhs_fn(ik, tp, r0),
                                start=(ik == 0 and tp == 0),
                                stop=(ik == n_k - 1 and tp == K * K - 1),
                            )
            for ci in range(4):
                for mt in range(2):
                    out_fn(mt, qh * 32 + ci * 8, pss[mt][ci])
            if boundary_hook is not None:
                boundary_hook()

    def emit_image_passes(img, boundary_hook=None, q2_mid_hook=None):
        """The four conv passes for one image (q2, q3, q4, concat)."""
        g0t = q0[img]

        for j, K in ((0, 3), (1, 5), (2, 7)):
            base = 3 - (K // 2)
            dst = qb[j]

            def rhs_m(ik, tp, r0, K=K, base=base):
                ky, kx = tp // K, tp % K
                return g0t[ik][
                    :, base + ky + r0 : base + ky + r0 + 8, base + kx : base + kx + 64
                ]

            def wr(mt, r0, ps_, dst=dst):
                if mt == 0:
                    nc.vector.tensor_copy(dst[0][:, 1 + r0 : 9 + r0, 1:65], ps_)
                else:
                    nc.scalar.copy(dst[1][:, 1 + r0 : 9 + r0, 1:65], ps_)

            hook = boundary_hook
            mid = q2_mid_hook if j == 0 else None
            if mid is not None:
                # run the mid hook after qh0 of this pass, main hook after qh1
                state = {"n": 0}

                def hook(state=state, mid=mid, bh=boundary_hook):
                    state["n"] += 1
                    if state["n"] == 1:
                        mid()
                    elif bh is not None:
                        bh()

            conv_pass(ap[f"w{j+2}"], 2, K, rhs_m, wr, hook)

        def rhs_cat(ik, tp, r0):
            g, k = ik // 2, ik % 2
            ky, kx = tp // 3, tp % 3
            if g == 0:
                return g0t[k][:, 2 + ky + r0 : 2 + ky + r0 + 8, 2 + kx : 2 + kx + 64]
            src = pb[img][k] if g == 4 else qb[g - 1][k]
            return src[:, ky + r0 : ky + r0 + 8, kx : kx + 64]

        # concat conv drain: write qkv f16; on the mt=1 chunk run the f16
        # content conv on the fresh qkv tiles, exp onto partition r0//8 of e8
        def wr_cat(mt, r0, ps_):
            if mt == 0:
                nc.vector.tensor_copy(qkv[0][:, 1 + r0 : 9 + r0, 1:65], ps_)
                return
            nc.scalar.copy(qkv[1][:, 1 + r0 : 9 + r0, 1:65], ps_)
            ci = r0 // 8
            cp = psum.tile([1, 8, 64], F32, tag="ps", name="cp")
            for k in range(2):
                nc.tensor.matmul(
                    cp, wcont16[k], qkv[k][:, 1 + r0 : 9 + r0, 1:65],
                    start=(k == 0), stop=(k == 1),
                )
            nc.scalar.activation(
                out=e_sb[:, r0 : r0 + 8, :], in_=cp, func=AF.Exp,
                bias=0.0, scale=1.0, accum_out=denc[:, ci : ci + 1],
            )

        conv_pass(ap["wcat"], 10, 3, rhs_cat, wr_cat, boundary_hook)

    # ---- attention tail part A: softmax normalization, e-broadcast, s-pool
    def tail_a(img):
        dent = work.tile([1, 1], F32, tag="dent", name="dent")
        nc.vector.tensor_reduce(
            dent, denc, axis=mybir.AxisListType.X, op=ALU.add
        )
        rden = work.tile([1, 1], F32, tag="rden", name="rden")
        nc.vector.reciprocal(rden, dent)
        ones_sc = work.tile([1, 128], BF16, tag="ones_sc", name="ones_sc")
        nc.vector.tensor_scalar_mul(ones_sc, onesb, rden)
        # broadcast ebc[p, n] = e[n] / den
        for ci in range(8):
            eb_ps = psum.tile([128, 8, 64], F32, tag="ps", name="eb_ps")
            nc.tensor.matmul(
                eb_ps, ones_sc, e_sb[:, ci * 8 : (ci + 1) * 8, :],
                start=True, stop=True,
            )
            nc.scalar.copy(ebc[:, ci * 8 : (ci + 1) * 8, :], eb_ps)
        # s[i, tap] = sum_n a_n * qkv[i, n+tap]: multiply + XY-reduce per
        # row-chunk, then reduce the chunk partials
        for ik in range(2):
            for tp in range(9):
                ky, kx = tp // 3, tp % 3
                for ch in range(8):
                    r0 = ch * 8
                    ttr = work.tile(
                        [128, 8, 64], F16, tag="ttr", bufs=2, name="ttr"
                    )
                    nc.vector.tensor_mul(
                        ttr, qkv[ik][:, ky + r0 : ky + r0 + 8, kx : kx + 64],
                        ebc[:, r0 : r0 + 8, :],
                    )
                    nc.vector.tensor_reduce(
                        sp_c[ik][:, tp, ch : ch + 1], ttr,
                        axis=mybir.AxisListType.XY, op=ALU.add,
                    )
            nc.vector.tensor_reduce(
                spart[ik], sp_c[ik], axis=mybir.AxisListType.X, op=ALU.add
            )
            nc.vector.tensor_copy(s16[ik], spart[ik])

    # ---- attention tail part B: pooled matvec, channel transform, proj conv
    def tail_b(img):
        pooled_ps = [
            psum.tile([128, 1], F32, tag="ps", name=f"pool_ps{mt}") for mt in range(2)
        ]
        for ik in range(2):
            for tp in range(9):
                wt = wring.tile([128, 256], F16, tag="wtap", bufs=8, name="wtc")
                nc.sync.dma_start(
                    out=wt, in_=ap["wch"][ik, tp].rearrange("m p c -> p m c")
                )
                for mt in range(2):
                    nc.tensor.matmul(
                        pooled_ps[mt], wt[:, mt * 128 : (mt + 1) * 128],
                        s16[ik][:, tp : tp + 1],
                        start=(ik == 0 and tp == 0), stop=(ik == 1 and tp == 8),
                    )
        pooled = []
        for mt in range(2):
            pl = work.tile([128, 1], F32, tag=f"pool{mt}", name="pl")
            nc.vector.tensor_copy(pl, pooled_ps[mt])
            pooled.append(pl)

        # channel transform (tiny, fp32)
        t_ps = psum.tile([32, 1], F32, tag="ps", name="t_ps")
        for k in range(2):
            nc.tensor.matmul(t_ps, wct1[k], pooled[k], start=(k == 0), stop=(k == 1))
        ts2 = work.tile([32, 2], F32, tag="ts2", name="ts2")
        t_sb = ts2[:, 0:1]
        nc.vector.tensor_scalar_add(t_sb, t_ps, ct1b)
        nc.vector.tensor_mul(ts2[:, 1:2], t_sb, t_sb)
        sums_ps = psum.tile([1, 2], F32, tag="ps", name="sums_ps")
        nc.tensor.matmul(sums_ps, onesf, ts2, start=True, stop=True)
        sums_sb = work.tile([1, 2], F32, tag="sums_sb", name="sums_sb")
        nc.vector.tensor_copy(sums_sb, sums_ps)
        bc_ps = psum.tile([32, 2], F32, tag="ps", name="bc_ps")
        nc.tensor.matmul(bc_ps, onesf2, sums_sb, start=True, stop=True)
        mean = work.tile([32, 1], F32, tag="mean", name="mean")
        nc.vector.tensor_scalar_mul(mean, bc_ps[:, 0:1], 1.0 / 32)
        mv = work.tile([32, 1], F32, tag="mv", name="mv")
        nc.vector.tensor_scalar_mul(mv, bc_ps[:, 1:2], 1.0 / 32)
        m2t = work.tile([32, 1], F32, tag="m2t", name="m2t")
        nc.vector.tensor_mul(m2t, mean, mean)
        var = work.tile([32, 1], F32, tag="var", name="var")
        nc.vector.tensor_sub(var, mv, m2t)
        sd = work.tile([32, 1], F32, tag="sd", name="sd")
        nc.scalar.activation(out=sd, in_=var, func=AF.Sqrt, bias=epsv, scale=1.0)
        rsd = work.tile([32, 1], F32, tag="rsd", name="rsd")
        nc.vector.reciprocal(rsd, sd)
        dt_ = work.tile([32, 1], F32, tag="dt", name="dt_")
        nc.vector.tensor_sub(dt_, t_sb, mean)
        tn = work.tile([32, 1], F32, tag="tn", name="tn")
        nc.vector.tensor_mul(tn, dt_, rsd)
        tact = work.tile([32, 1], F32, tag="tact", name="tact")
        nc.scalar.activation(out=tact, in_=tn, func=AF.Relu, bias=lnb, scale=lng)

        projs = []
        for mt in range(2):
            cw_ps = psum.tile([128, 1], F32, tag="ps", name="cw_ps")
            nc.tensor.matmul(
                cw_ps, wct2[:, mt * 128 : (mt + 1) * 128], tact, start=True, stop=True
            )
            cw = work.tile([128, 1], F32, tag=f"cw{mt}", name="cw")
            nc.vector.tensor_scalar_add(cw, cw_ps, ct2b[mt])
            pj = work.tile([128, 256], F16, tag=f"projs{mt}", name="pj")
            nc.vector.tensor_scalar_mul(pj, wproj[mt], cw)
            projs.append(pj)

        # out = proj(qkv * cw)  (cw folded into proj weights)
        for mt in range(2):
            for ci in range(8):
                r0 = ci * 8
                po = psum.tile([128, 8, 64], F32, tag="ps", name="po")
                for k in range(2):
                    nc.tensor.matmul(
                        po,
                        projs[k][:, mt * 128 : (mt + 1) * 128],
                        qkv[k][:, 1 + r0 : 9 + r0, 1:65],
                        start=(k == 0), stop=(k == 1),
                    )
                ost = work.tile([128, 8, 64], F32, tag="ost", bufs=2, name="ost")
                nc.scalar.copy(ost, po)
                nc.sync.dma_start(
                    out=ap["out"][img, mt * 128 : (mt + 1) * 128, r0 : r0 + 8, :],
                    in_=ost,
                )

    # shared tail tiles
    e_sb = work.tile([1, 64, 64], BF16, tag="e_sb", name="e_sb")
    denc = work.tile([1, 8], F32, tag="denc", name="denc")
    ebc = work.tile([128, 64, 64], F16, tag="ebc", name="ebc")
    sp_c = [
        work.tile([128, 9, 8], F32, tag=f"sp_c{ik}", name="sp_c") for ik in range(2)
    ]
    spart = [
        work.tile([128, 9], F32, tag=f"spart{ik}", name="spart") for ik in range(2)
    ]
    s16 = [work.tile([128, 9], F16, tag=f"s16_{ik}", name="s16") for ik in range(2)]

    # ---- schedule ----
    ph1 = [Ph1(im) for im in range(BPC)]
    ph1[0].emit_dmas()
    ph1[0].step(16)
    ph1[1].emit_dmas()
    # image 0 passes; image 1's phase-1 slots in at the 8 qh boundaries
    emit_image_passes(0, boundary_hook=lambda: ph1[1].step(2))
    tail_a(0)
    # image 1 passes; image 0's tail_b (matvec+ct+proj) runs inside the q2
    # pass at its qh boundary so the PE never waits on image 0's DVE pooling
    emit_image_passes(1, q2_mid_hook=lambda: tail_b(0))
    tail_a(1)
    tail_b(1)
    ctx.close()


def build():
    nc = bass.Bass("TRN2", target_bir_lowering=False, debug=False)
    shapes = {
        "x": ([BPC, C, H, W], F16),
        "wred": ([2, 1, 128, 128], F16),
        "bred": ([128, 1], F32),
        "w2": ([2, 9, 2, 128, 128], F16),
        "w3": ([2, 25, 2, 128, 128], F16),
        "w4": ([2, 49, 2, 128, 128], F16),
        "wcat": ([10, 9, 2, 128, 128], F16),
        "wch": ([2, 9, 2, 128, 128], F16),
        "wcont16": ([2, 128, 1], F16),
        "wproj": ([2, 1, 128, 256], F16),
        "wct1": ([2, 1, 128, 32], F32),
        "ct1b": ([32, 1], F32),
        "wct2": ([1, 1, 32, 256], F32),
        "ct2b": ([2, 128, 1], F32),
        "lng": ([32, 1], F32),
        "lnb": ([32, 1], F32),
    }
    ap = {
        k: nc.dram_tensor(k, shp, dt, kind="ExternalInput").ap()
        for k, (shp, dt) in shapes.items()
    }
    ap["out"] = nc.dram_tensor("out", [BPC, C, H2, W2], F32, kind="ExternalOutput").ap()
    with tile.TileContext(nc) as tc:
        _emit(nc, tc, ap)
    return nc


_CACHED_NC = {}


def _install_trace_hook():
    """The image's antenv lacks axon_hooks; shim it and register the boot's
    ctypes NTFF hook so trace=True works.  Also neutralize the S3 artifact
    upload (no bucket access here)."""
    import types
    import antenv

    if "antenv.axon_hooks" not in sys.modules:
        mod = types.ModuleType("antenv.axon_hooks")
        mod._hook = None
        def set_axon_ntff_profile_hook(h):
            mod._hook = h
        def get_axon_ntff_profile_hook():
            return mod._hook
        mod.set_axon_ntff_profile_hook = set_axon_ntff_profile_hook
        mod.get_axon_ntff_profile_hook = get_axon_ntff_profile_hook
        sys.modules["antenv.axon_hooks"] = mod
        antenv.axon_hooks = mod
        from trn_agent_boot.trn_boot import _ntff_profile_via_ctypes
        mod.set_axon_ntff_profile_hook(
            _ntff_profile_via_ctypes("/opt/axon/libaxon_pjrt.so")
        )
        bass_utils.upload_artifacts = lambda tmpdir: tmpdir


def run(inputs, debug=False, trace=False):
    if trace:
        _install_trace_hook()
    if "nc" not in _CACHED_NC:
        _CACHED_NC["nc"] = build()
    nc = _CACHED_NC["nc"]
    d = _prep_inputs(inputs)
    x_f16 = np.asarray(inputs["x"], np.float32).astype(np.float16)
    in_maps = []
    for c in range(N_CORES):
        m = dict(d)
        m["x"] = np.ascontiguousarray(x_f16[c * BPC : (c + 1) * BPC])
        in_maps.append(m)
    res = bass_utils.run_bass_kernel_spmd(
        nc, in_maps, core_ids=list(range(N_CORES)), trace=trace
    )
    out = np.concatenate([res.results[c]["out"] for c in range(N_CORES)], axis=0)
    return out, res


def kernel(**inputs):
    out, _ = run(inputs)
    return out


# revision 12
# speedup vs baseline: 1.0879x; 1.0361x over previous
"""Trainium2 Bass kernel for nn_Dwtpool (dense_cnn).

Reference graph (per image, C=256, 128x128 input):
  p    = maxpool2x2(x)                          -> [256, 64, 64]
  r    = ReLU(BN(conv1x1(x, reduce_w)))         -> [ 64,128,128]
  M    = haar_dwt(r) * 2  (stored unscaled)     -> [256, 64, 64]
  q2..q4 = conv{3,5,7}(0.5*M)                   -> [256, 64, 64] each
  qkv  = conv3x3(concat[0.5*M, q1..q4, p])      -> [256, 64, 64]
  att  = softmax_spatial(conv1x1(qkv)); pooled = sum_n ch(qkv)_c,n * att_n
  cw   = ct2(ReLU(LN(ct1(pooled))))             -> [256]
  out  = conv1x1(qkv * cw, proj_w)              -> [256, 64, 64]

Strategy: data-parallel over batch (16 images / 8 cores = 2 per core), fp16
trunk (same PE speed as bf16, 8x less noise; the softmax path amplifies qkv
noise ~3x into the output so 16-bit stays mandatory).  All convs are
tap-accumulated f16 matmuls into PSUM.  Algebraic cuts vs the naive graph:
  * q1 (1x1 conv) is folded into concat-conv group 0 on the host
    (conv3x3(W1, conv1x1(c1, x)) == conv3x3(W1 . c1, x)), removing one conv
    and one concat group.
  * the channel conv never materializes: pooled = sum_n a_n*conv(qkv)[:,n]
    == wch . s where s[i,tap] = sum_n a_n * qkv[i, n+tap], computed on the
    DVE with fused tensor_tensor_reduce, then an 18-matmul matvec.
  * content logits are a f16 matmul on the qkv tiles (replicated onto 8
    partitions so exp/accum runs 8-wide), softmax denominator folded into
    the e-broadcast ones-vector.
Phase-1 (reduce+DWT+maxpool) of image 1 is interleaved at the conv-pass PSUM
boundaries of image 0, and image 0's attention tail + proj run inside image
1's first conv pass, keeping the PE stream dense.
"""
import os
import sys

for _p in ("/opt/trn_rl_repo", os.path.expanduser("~/.axon_site/_ro/trn_rl_repo")):
    if os.path.isdir(_p) and _p not in sys.path:
        sys.path.append(_p)

import numpy as np
import ml_dtypes
from contextlib import ExitStack

import concourse.bass as bass
import concourse.tile as tile
from concourse import mybir
from concourse import bass_utils

BF16 = mybir.dt.bfloat16
F16 = mybir.dt.float16
F32 = mybir.dt.float32
AF = mybir.ActivationFunctionType
ALU = mybir.AluOpType

B, C, H, W = 16, 256, 128, 128
H2, W2 = 64, 64
N_CORES = 8
BPC = B // N_CORES  # images per core
EPS = 1e-5

# ---------------------------------------------------------------------------
# walrus CoreV3 rejects instructions with more than a couple of sync waits;
# Tile's exit drain accumulates one wait per processor used.  Split the waits
# across a chain of drain instructions (sync engine executes them in order).
# ---------------------------------------------------------------------------
import bass_rust as _br
import concourse.tile as _tile_mod

def _split_drain_and_barrier(self, tick_clock, wait_clock):
    nc = self.nc
    drain_inst = nc.sync.drain()
    wait_clock.add_sem_waits(
        drain_inst.ins, _tile_mod.ScopedClock({None: tick_clock.global_clock})
    )
    W_ = list(drain_inst.ins.sync_info.on_wait)
    if len(W_) > 1:
        drain_inst.ins.sync_info.on_wait = W_[:1]
        for i in range(1, len(W_)):
            extra = nc.sync.drain()
            extra.ins.sync_info = _br.SyncInfo(on_wait=W_[i : i + 1], on_update=[])
    nc.all_engine_barrier()
    assert self.sems is not None
    popped = nc._tile_sem_poison_stack.pop()
    assert popped is self._sem_poison
    nc.clear_and_free_semaphores(list(self.sems.allocated().values()))
    nc.all_engine_barrier()

tile.TileContext._drain_and_barrier = _split_drain_and_barrier

# Same hardware limit applies to scheduled body instructions (max 2 sync waits
# per instruction).  Before lowering, move excess waits onto injected NOPs on
# the same engine.
_MAX_W = 1
_orig_lower_ordered = tile.TileContext._lower_ordered_insts

def _lower_with_wait_split(self, ordered):
    for _bb, insts in ordered.items():
        out = []
        for inst in insts:
            si = getattr(inst, "sync_info", None)
            if si is not None and len(si.on_wait) > _MAX_W:
                wl = list(si.on_wait)
                extra, keep = wl[:-_MAX_W], wl[-_MAX_W:]
                si.on_wait = keep
                for i in range(0, len(extra), _MAX_W):
                    nop = mybir.InstNoOp(
                        name=f"{inst.name}-wsplit{i}",
                        sync_info=mybir.SyncInfo(
                            on_wait=extra[i : i + _MAX_W], on_update=[]
                        ),
                        bass_nofuse=True,
                        engine=inst.engine,
                    )
                    out.append(nop)
            out.append(inst)
        insts[:] = out
    return _orig_lower_ordered(self, ordered)

tile.TileContext._lower_ordered_insts = _lower_with_wait_split


# ---------------------------------------------------------------------------
# host-side weight packing
# ---------------------------------------------------------------------------
def _pack_conv(w, scale=1.0):
    """[O, I, K, K] -> [n_kt, K*K, kt_size, O]  (lhsT blocks per ktile/tap)."""
    O, I, K, _ = w.shape
    kt = 128 if I >= 128 else I
    nkt = I // kt
    a = (np.asarray(w, np.float32) * scale).transpose(1, 2, 3, 0)  # [I,K,K,O]
    a = a.reshape(nkt, kt, K, K, O).transpose(0, 2, 3, 1, 4)
    return np.ascontiguousarray(a.reshape(nkt, K * K, kt, O))


def _f16(a):
    return np.asarray(a).astype(np.float16)


def _pack_conv5(w, scale=1.0):
    """[O, I, K, K] -> [n_kt, K*K, n_mt, kt, 128]: per (ktile, tap, mtile)
    contiguous lhsT blocks for the streaming conv passes."""
    a = _pack_conv(w, scale)  # [nkt, KK, kt, O]
    nkt, kk, kt, O = a.shape
    return np.ascontiguousarray(
        a.reshape(nkt, kk, kt, O // 128, 128).transpose(0, 1, 3, 2, 4)
    )


def _prep_inputs(inp):
    """Full problem inputs -> dict of packed host arrays (shared by cores)."""
    d = {}
    # reduce conv: fold BN, duplicate output channels to fill 128 partitions
    sc = np.asarray(inp["bn_g"], np.float32) / np.sqrt(
        np.asarray(inp["bn_var"], np.float32) + EPS
    )
    w_red = np.asarray(inp["reduce_w"], np.float32)[:, :, 0, 0] * sc[:, None]  # [64,256]
    b_red = (
        np.asarray(inp["reduce_b"], np.float32) - np.asarray(inp["bn_mean"], np.float32)
    ) * sc + np.asarray(inp["bn_b"], np.float32)
    w_red2 = np.concatenate([w_red, w_red], axis=0)  # [128, 256]
    d["wred"] = _f16(_pack_conv(w_red2[:, :, None, None]))  # [2,1,128,128]
    d["bred"] = np.concatenate([b_red, b_red])[:, None].astype(np.float32)  # [128,1]

    # DWT-branch convs: input is M = 2*qkv0, so fold the 0.5 into weights
    d["w2"] = _f16(_pack_conv5(inp["conv2_w"], 0.5))
    d["w3"] = _f16(_pack_conv5(inp["conv3_w"], 0.5))
    d["w4"] = _f16(_pack_conv5(inp["conv4_w"], 0.5))
    # concat conv groups [qkv0(=0.5*M), q2, q3, q4, p]; the q1 group is folded
    # into group 0: conv3x3(W1, conv1x1(c1, qkv0)) == conv3x3(W1 . c1, qkv0)
    wcat = np.asarray(inp["conv1x1_w"], np.float32)
    conv1 = np.asarray(inp["conv1_w"], np.float32)[:, :, 0, 0]  # [256,256]
    g0 = wcat[:, 0:256] + np.einsum("ocyx,ci->oiyx", wcat[:, 256:512], conv1)
    packs = [_pack_conv5(g0, 0.5)]
    for g in range(2, 6):
        packs.append(_pack_conv5(wcat[:, g * 256 : (g + 1) * 256]))
    d["wcat"] = _f16(np.concatenate(packs, axis=0))  # [10,9,2,128,128]
    d["wch"] = _f16(_pack_conv5(inp["channel_conv_w"]))  # [2,9,2,128,128]
    wc = np.asarray(inp["conv_w"], np.float32)[0, :, 0, 0]  # [256]
    d["wcont16"] = _f16(wc.reshape(2, 128, 1))  # [2,128,1]
    d["wproj"] = _f16(_pack_conv(inp["proj_w"]))  # [2,1,128,256]
    d["wct1"] = _pack_conv(inp["ct1_w"]).astype(np.float32)  # [2,1,128,32]
    d["ct1b"] = np.asarray(inp["ct1_b"], np.float32)[:, None]  # [32,1]
    d["wct2"] = _pack_conv(inp["ct2_w"]).astype(np.float32)  # [1,1,32,256]
    d["ct2b"] = np.asarray(inp["ct2_b"], np.float32).reshape(2, 128, 1)
    d["ident"] = np.eye(128, dtype=np.float16)
    d["lng"] = np.asarray(inp["ln_g"], np.float32)[:, None]
    d["lnb"] = np.asarray(inp["ln_b"], np.float32)[:, None]
    return d


# ---------------------------------------------------------------------------
# kernel body
# ---------------------------------------------------------------------------
def _emit(nc, tc, ap):
    ctx = ExitStack()
    consts = ctx.enter_context(tc.tile_pool(name="consts", bufs=1))
    acts = ctx.enter_context(tc.tile_pool(name="acts", bufs=1))
    wring = ctx.enter_context(tc.tile_pool(name="wring", bufs=1))
    work = ctx.enter_context(tc.tile_pool(name="work", bufs=1))
    psum = ctx.enter_context(tc.tile_pool(name="psum", bufs=8, space="PSUM"))

    def cst(name, shape, dtype, src):
        t = consts.tile(shape, dtype, tag=name, name=name)
        nc.sync.dma_start(out=t, in_=src)
        return t

    wred = [cst(f"wred{k}", [128, 128], F16, ap["wred"][k, 0]) for k in range(2)]
    bred = cst("bred", [128, 1], F32, ap["bred"])
    wcont16 = [cst(f"wcont16{k}", [128, 1], F16, ap["wcont16"][k]) for k in range(2)]
    wproj = [cst(f"wproj{k}", [128, 256], F16, ap["wproj"][k, 0]) for k in range(2)]
    wct1 = [cst(f"wct1{k}", [128, 32], F32, ap["wct1"][k, 0]) for k in range(2)]
    wct2 = cst("wct2", [32, 256], F32, ap["wct2"][0, 0])
    ct1b = cst("ct1b", [32, 1], F32, ap["ct1b"])
    ct2b = [cst(f"ct2b{k}", [128, 1], F32, ap["ct2b"][k]) for k in range(2)]
    ident = cst("ident", [128, 128], F16, ap["ident"])
    lng = cst("lng", [32, 1], F32, ap["lng"])
    lnb = cst("lnb", [32, 1], F32, ap["lnb"])

    sigma = consts.tile([128, 1], F32, tag="sigma", name="sigma")
    nc.vector.memset(sigma[0:64, :], 1.0)
    nc.vector.memset(sigma[64:128, :], -1.0)
    epsv = consts.tile([32, 1], F32, tag="epsv", name="epsv")
    nc.vector.memset(epsv, EPS)
    onesb = consts.tile([1, 128], BF16, tag="onesb", name="onesb")
    nc.vector.memset(onesb, 1.0)
    onesf = consts.tile([32, 1], F32, tag="onesf", name="onesf")
    nc.vector.memset(onesf, 1.0)
    onesf2 = consts.tile([1, 32], F32, tag="onesf2", name="onesf2")
    nc.vector.memset(onesf2, 1.0)

    # ---- activation buffers (f16).  q0/pb are per-image (phase-1 of image
    # i+1 overlaps image i's conv passes); qb/qkv are shared (WAR deps order
    # them behind the previous image's reads, which is late enough).
    def padbuf(name, hw):
        return acts.tile([128, hw, hw], F16, tag=name, name=name)

    q0 = [[padbuf(f"q0_{im}_{k}", 70) for k in range(2)] for im in range(BPC)]
    pb = [[padbuf(f"p_{im}_{k}", 66) for k in range(2)] for im in range(BPC)]
    qb = [[padbuf(f"q{j}_{k}", 66) for k in range(2)] for j in range(3)]
    qkv = [padbuf(f"qkv_{k}", 66) for k in range(2)]

    # zero the halo borders once (interior is fully overwritten per image)
    for t in [t_ for im in range(BPC) for t_ in q0[im]]:
        nc.gpsimd.memset(t[:, 0:3, :], 0.0)
        nc.gpsimd.memset(t[:, 67:70, :], 0.0)
        nc.gpsimd.memset(t[:, 3:67, 0:3], 0.0)
        nc.gpsimd.memset(t[:, 3:67, 67:70], 0.0)
    for t in [t_ for im in range(BPC) for t_ in pb[im]] + [
        t_ for j in range(3) for t_ in qb[j]
    ] + qkv:
        nc.gpsimd.memset(t[:, 0:1, :], 0.0)
        nc.gpsimd.memset(t[:, 65:66, :], 0.0)
        nc.gpsimd.memset(t[:, 1:65, 0:1], 0.0)
        nc.gpsimd.memset(t[:, 1:65, 65:66], 0.0)

    # ---- phase 1: stream x -> reduce conv+ReLU -> DWT -> M;  maxpool -> p
    class Ph1:
        def __init__(self, img):
            self.img = img
            self.xts = []
            self.sc = 0

        def emit_dmas(self):
            for sc_ in range(16):
                pair = []
                for k in range(2):
                    xt = work.tile(
                        [128, 8, 128], F16, tag=f"x{k}", bufs=3, name=f"xt{k}"
                    )
                    src = ap["x"][
                        self.img, k * 128 : (k + 1) * 128, sc_ * 8 : sc_ * 8 + 8, :
                    ]
                    nc.sync.dma_start(out=xt[:, 0:4, :], in_=src[:, 0:4, :])
                    nc.sync.dma_start(out=xt[:, 4:8, :], in_=src[:, 4:8, :])
                    pair.append(xt)
                self.xts.append(pair)

        def step(self, n=1):
            for _ in range(n):
                if self.sc < 16:
                    self._sc(self.sc)
                    self.sc += 1

        def _sc(self, sc_):
            xts = self.xts[sc_]
            orow = sc_ * 4  # 8 input rows -> 4 output rows
            rch = work.tile([128, 8, 128], F16, tag="rch", bufs=2, name="rch")
            for sub in range(2):
                ps = psum.tile([128, 4, 128], F32, tag="ps", name="ps_r")
                for k in range(2):
                    nc.tensor.matmul(
                        ps, wred[k], xts[k][:, sub * 4 : sub * 4 + 4, :],
                        start=(k == 0), stop=(k == 1),
                    )
                nc.scalar.activation(
                    out=rch[:, sub * 4 : sub * 4 + 4, :], in_=ps, func=AF.Relu,
                    bias=bred, scale=1.0,
                )
            rv = rch.rearrange("p (a two) (c cp) -> p a two c cp", two=2, cp=2)
            a_, b_ = rv[:, :, 0, :, 0], rv[:, :, 0, :, 1]
            c_, d_ = rv[:, :, 1, :, 0], rv[:, :, 1, :, 1]
            u = work.tile([128, 4, 64], F32, tag="u", bufs=2, name="u")
            v = work.tile([128, 4, 64], F32, tag="v", bufs=2, name="v")
            s_ = work.tile([128, 4, 64], F32, tag="s", bufs=2, name="s_")
            t_ = work.tile([128, 4, 64], F32, tag="t", bufs=2, name="t_")
            nc.vector.tensor_add(u, a_, b_)
            nc.vector.tensor_add(v, c_, d_)
            nc.vector.tensor_sub(s_, a_, b_)
            nc.vector.tensor_sub(t_, c_, d_)
            sv = work.tile([128, 4, 64], F32, tag="sv", bufs=2, name="sv")
            st = work.tile([128, 4, 64], F32, tag="st", bufs=2, name="st")
            # sigma-scale on the Scalar engine to unload DVE
            nc.scalar.activation(out=sv, in_=v, func=AF.Copy, scale=sigma)
            nc.scalar.activation(out=st, in_=t_, func=AF.Copy, scale=sigma)
            myq0 = q0[self.img]
            nc.vector.tensor_add(myq0[0][:, 3 + orow : 7 + orow, 3:67], u, sv)
            nc.vector.tensor_add(myq0[1][:, 3 + orow : 7 + orow, 3:67], s_, st)
            for k in range(2):
                xv = xts[k].rearrange("p (a two) (c cp) -> p a two c cp", two=2, cp=2)
                xa, xb = xv[:, :, 0, :, 0], xv[:, :, 0, :, 1]
                xc, xd = xv[:, :, 1, :, 0], xv[:, :, 1, :, 1]
                m1 = work.tile([128, 4, 64], F16, tag="m1", bufs=2, name="m1")
                m2 = work.tile([128, 4, 64], F16, tag="m2", bufs=2, name="m2")
                nc.vector.tensor_max(m1, xa, xb)
                nc.vector.tensor_max(m2, xc, xd)
                nc.vector.tensor_max(pb[self.img][k][:, 1 + orow : 5 + orow, 1:65], m1, m2)

    # ---- conv pass: accumulate over (ktile, tap) into 8 psum banks
    def conv_pass(wdram, n_k, K, rhs_fn, out_fn, boundary_hook=None):
        for qh in range(2):
            pss = [
                [psum.tile([128, 8, 64], F32, tag="ps", name="ps_c") for _ in range(4)]
                for _ in range(2)
            ]
            for ik in range(n_k):
                for tp in range(K * K):
                    wt = wring.tile([128, 256], F16, tag="wtap", bufs=8, name="wt")
                    nc.sync.dma_start(
                        out=wt, in_=wdram[ik, tp].rearrange("m p c -> p m c")
                    )
                    for mt in range(2):
                        lhsT = wt[:, mt * 128 : (mt + 1) * 128]
                        for ci in range(4):
                            r0 = qh * 32 + ci * 8
                            nc.tensor.matmul(
                                pss[mt][ci], lhsT, rhs_fn(ik, tp, r0),
                                start=(ik == 0 and tp == 0),
                                stop=(ik == n_k - 1 and tp == K * K - 1),
                            )
            for ci in range(4):
                for mt in range(2):
                    out_fn(mt, qh * 32 + ci * 8, pss[mt][ci])
            if boundary_hook is not None:
                boundary_hook()

    def emit_image_passes(img, boundary_hook=None):
        """The four conv passes for one image (q2, q3, q4, concat)."""
        g0t = q0[img]

        for j, K in ((0, 3), (1, 5), (2, 7)):
            base = 3 - (K // 2)
            dst = qb[j]

            def rhs_m(ik, tp, r0, K=K, base=base):
                ky, kx = tp // K, tp % K
                return g0t[ik][
                    :, base + ky + r0 : base + ky + r0 + 8, base + kx : base + kx + 64
                ]

            def wr(mt, r0, ps_, dst=dst):
                if mt == 0:
                    nc.vector.tensor_copy(dst[0][:, 1 + r0 : 9 + r0, 1:65], ps_)
                else:
                    nc.scalar.copy(dst[1][:, 1 + r0 : 9 + r0, 1:65], ps_)

            conv_pass(ap[f"w{j+2}"], 2, K, rhs_m, wr, boundary_hook)

        def rhs_cat(ik, tp, r0):
            g, k = ik // 2, ik % 2
            ky, kx = tp // 3, tp % 3
            if g == 0:
                return g0t[k][:, 2 + ky + r0 : 2 + ky + r0 + 8, 2 + kx : 2 + kx + 64]
            src = pb[img][k] if g == 4 else qb[g - 1][k]
            return src[:, ky + r0 : ky + r0 + 8, kx : kx + 64]

        # concat conv drain: write qkv f16; on the mt=1 chunk run the f16
        # content conv on the fresh qkv tiles, exp onto partition r0//8 of e8
        def wr_cat(mt, r0, ps_):
            if mt == 0:
                nc.vector.tensor_copy(qkv[0][:, 1 + r0 : 9 + r0, 1:65], ps_)
                return
            nc.scalar.copy(qkv[1][:, 1 + r0 : 9 + r0, 1:65], ps_)
            ci = r0 // 8
            cp = psum.tile([1, 8, 64], F32, tag="ps", name="cp")
            for k in range(2):
                nc.tensor.matmul(
                    cp, wcont16[k], qkv[k][:, 1 + r0 : 9 + r0, 1:65],
                    start=(k == 0), stop=(k == 1),
                )
            nc.scalar.activation(
                out=e66i[:, 1 + r0 : 9 + r0, 1:65], in_=cp, func=AF.Exp,
                bias=0.0, scale=1.0, accum_out=denc[:, ci : ci + 1],
            )

        conv_pass(ap["wcat"], 10, 3, rhs_cat, wr_cat, boundary_hook)

    # ---- attention tail part A: softmax denominator, then the attention
    # pooling s[i,tap] = sum_n e_n * qkv[i, n+tap] entirely on the PE:
    # transpose qkv spatial->partition tile by tile, contract against 9
    # shifted (unnormalized) e-maps gathered by strided DMA, normalize by
    # 1/den when casting s to f16, transpose s back to channel-major.
    def tail_a(img):
        dent = work.tile([1, 1], F32, tag="dent", name="dent")
        nc.vector.tensor_reduce(
            dent, denc, axis=mybir.AxisListType.X, op=ALU.add
        )
        rden = work.tile([1, 1], F32, tag="rden", name="rden")
        nc.vector.reciprocal(rden, dent)
        rden_bf = work.tile([1, 1], BF16, tag="rden_bf", name="rden_bf")
        nc.vector.tensor_copy(rden_bf, rden)
        rd_ps = psum.tile([128, 1], F32, tag="ps", name="rd_ps")
        nc.tensor.matmul(rd_ps, onesb, rden_bf, start=True, stop=True)
        rdsb = work.tile([128, 1], F32, tag="rdsb", name="rdsb")
        nc.vector.tensor_copy(rdsb, rd_ps)
        # 9 shifted views of the padded e-grid, spatial-on-partitions.
        # SBUF partition dims can't alias free-dim strides, so bounce the
        # flat e-grid through DRAM scratch and gather from there.
        nc.sync.dma_start(out=ap["e_scr"], in_=e66_sb)
        aT = work.tile([128, 35, 9], BF16, tag="aT", name="aT")
        for k9 in range(9):
            ky, kx = k9 // 3, k9 % 3
            off = 201 - 66 * ky - kx
            nc.sync.dma_start(
                out=aT[:, :, k9],
                in_=ap["e_scr"][:, off : off + 4480].rearrange(
                    "o (t p) -> (o p) t", p=128
                ),
            )
        s_acc = work.tile([9, 256], F32, tag="s_acc", name="s_acc")
        qf = [qkv[ik].rearrange("p a b -> p (a b)") for ik in range(2)]
        for t in range(35):
            w_t = 128 if t < 34 else 4356 - 34 * 128
            qpT = work.tile([128, 2, 128], BF16, tag="qpT", bufs=4, name="qpT")
            for ik in range(2):
                tp_ps = psum.tile([w_t, 128], F16, tag="ps", name="tp_ps")
                nc.tensor.transpose(
                    tp_ps, qf[ik][:, t * 128 : t * 128 + w_t], ident
                )
                nc.scalar.copy(qpT[0:w_t, ik, :], tp_ps)
            smm = psum.tile([9, 256], F32, tag="ps", name="smm")
            nc.tensor.matmul(
                smm, aT[:, t, :], qpT.rearrange("p a b -> p (a b)"),
                start=True, stop=True,
            )
            if t == 0:
                nc.vector.tensor_copy(s_acc, smm)
            else:
                nc.vector.tensor_add(s_acc, s_acc, smm)
        s_sb = work.tile([9, 256], F16, tag="s_sb", name="s_sb")
        nc.scalar.activation(
            out=s_sb, in_=s_acc, func=AF.Copy, scale=rdsb[0:9, :]
        )
        for ik in range(2):
            st_ps = psum.tile([128, 9], F16, tag="ps", name="st_ps")
            nc.tensor.transpose(
                st_ps, s_sb[:, ik * 128 : (ik + 1) * 128], ident[0:9, 0:9]
            )
            nc.scalar.copy(s16[ik], st_ps)

    # ---- attention tail part B: pooled matvec, channel transform, proj conv
    def tail_b(img):
        pooled_ps = [
            psum.tile([128, 1], F32, tag="ps", name=f"pool_ps{mt}") for mt in range(2)
        ]
        for ik in range(2):
            for tp in range(9):
                wt = wring.tile([128, 256], F16, tag="wtap", bufs=8, name="wtc")
                nc.sync.dma_start(
                    out=wt, in_=ap["wch"][ik, tp].rearrange("m p c -> p m c")
                )
                for mt in range(2):
                    nc.tensor.matmul(
                        pooled_ps[mt], wt[:, mt * 128 : (mt + 1) * 128],
                        s16[ik][:, tp : tp + 1],
                        start=(ik == 0 and tp == 0), stop=(ik == 1 and tp == 8),
                    )
        pooled = []
        for mt in range(2):
            pl = work.tile([128, 1], F32, tag=f"pool{mt}", name="pl")
            nc.vector.tensor_copy(pl, pooled_ps[mt])
            pooled.append(pl)

        # channel transform (tiny, fp32)
        t_ps = psum.tile([32, 1], F32, tag="ps", name="t_ps")
        for k in range(2):
            nc.tensor.matmul(t_ps, wct1[k], pooled[k], start=(k == 0), stop=(k == 1))
        ts2 = work.tile([32, 2], F32, tag="ts2", name="ts2")
        t_sb = ts2[:, 0:1]
        nc.vector.tensor_scalar_add(t_sb, t_ps, ct1b)
        nc.vector.tensor_mul(ts2[:, 1:2], t_sb, t_sb)
        sums_ps = psum.tile([1, 2], F32, tag="ps", name="sums_ps")
        nc.tensor.matmul(sums_ps, onesf, ts2, start=True, stop=True)
        sums_sb = work.tile([1, 2], F32, tag="sums_sb", name="sums_sb")
        nc.vector.tensor_copy(sums_sb, sums_ps)
        bc_ps = psum.tile([32, 2], F32, tag="ps", name="bc_ps")
        nc.tensor.matmul(bc_ps, onesf2, sums_sb, start=True, stop=True)
        mean = work.tile([32, 1], F32, tag="mean", name="mean")
        nc.vector.tensor_scalar_mul(mean, bc_ps[:, 0:1], 1.0 / 32)
        mv = work.tile([32, 1], F32, tag="mv", name="mv")
        nc.vector.tensor_scalar_mul(mv, bc_ps[:, 1:2], 1.0 / 32)
        m2t = work.tile([32, 1], F32, tag="m2t", name="m2t")
        nc.vector.tensor_mul(m2t, mean, mean)
        var = work.tile([32, 1], F32, tag="var", name="var")
        nc.vector.tensor_sub(var, mv, m2t)
        sd = work.tile([32, 1], F32, tag="sd", name="sd")
        nc.scalar.activation(out=sd, in_=var, func=AF.Sqrt, bias=epsv, scale=1.0)
        rsd = work.tile([32, 1], F32, tag="rsd", name="rsd")
        nc.vector.reciprocal(rsd, sd)
        dt_ = work.tile([32, 1], F32, tag="dt", name="dt_")
        nc.vector.tensor_sub(dt_, t_sb, mean)
        tn = work.tile([32, 1], F32, tag="tn", name="tn")
        nc.vector.tensor_mul(tn, dt_, rsd)
        tact = work.tile([32, 1], F32, tag="tact", name="tact")
        nc.scalar.activation(out=tact, in_=tn, func=AF.Relu, bias=lnb, scale=lng)

        projs = []
        for mt in range(2):
            cw_ps = psum.tile([128, 1], F32, tag="ps", name="cw_ps")
            nc.tensor.matmul(
                cw_ps, wct2[:, mt * 128 : (mt + 1) * 128], tact, start=True, stop=True
            )
            cw = work.tile([128, 1], F32, tag=f"cw{mt}", name="cw")
            nc.vector.tensor_scalar_add(cw, cw_ps, ct2b[mt])
            pj = work.tile([128, 256], F16, tag=f"projs{mt}", name="pj")
            nc.vector.tensor_scalar_mul(pj, wproj[mt], cw)
            projs.append(pj)

        # out = proj(qkv * cw)  (cw folded into proj weights)
        for mt in range(2):
            for ci in range(8):
                r0 = ci * 8
                po = psum.tile([128, 8, 64], F32, tag="ps", name="po")
                for k in range(2):
                    nc.tensor.matmul(
                        po,
                        projs[k][:, mt * 128 : (mt + 1) * 128],
                        qkv[k][:, 1 + r0 : 9 + r0, 1:65],
                        start=(k == 0), stop=(k == 1),
                    )
                ost = work.tile([128, 8, 64], F32, tag="ost", bufs=2, name="ost")
                nc.scalar.copy(ost, po)
                nc.sync.dma_start(
                    out=ap["out"][img, mt * 128 : (mt + 1) * 128, r0 : r0 + 8, :],
                    in_=ost,
                )

    # shared tail tiles.  e66_sb: e embedded at offset 134 on a 66-wide,
    # zero-bordered grid (margins make all 9 shifted DMA gathers in-range).
    e66_sb = work.tile([1, 4736], BF16, tag="e66", name="e66")
    nc.gpsimd.memset(e66_sb, 0.0)
    e66i = e66_sb[:, 134 : 134 + 4488].rearrange("o (r c) -> o r c", c=66)
    denc = work.tile([1, 8], F32, tag="denc", name="denc")
    s16 = [work.tile([128, 9], F16, tag=f"s16_{ik}", name="s16") for ik in range(2)]

    # ---- schedule ----
    ph1 = [Ph1(im) for im in range(BPC)]
    ph1[0].emit_dmas()
    ph1[0].step(16)
    ph1[1].emit_dmas()
    # image 0 passes; image 1's phase-1 slots in at the 8 qh boundaries
    emit_image_passes(0, boundary_hook=lambda: ph1[1].step(2))
    tail_a(0)
    tail_b(0)
    emit_image_passes(1)
    tail_a(1)
    tail_b(1)
    ctx.close()


def build():
    nc = bass.Bass("TRN2", target_bir_lowering=False, debug=False)
    shapes = {
        "x": ([BPC, C, H, W], F16),
        "wred": ([2, 1, 128, 128], F16),
        "bred": ([128, 1], F32),
        "w2": ([2, 9, 2, 128, 128], F16),
        "w3": ([2, 25, 2, 128, 128], F16),
        "w4": ([2, 49, 2, 128, 128], F16),
        "wcat": ([10, 9, 2, 128, 128], F16),
        "wch": ([2, 9, 2, 128, 128], F16),
        "wcont16": ([2, 128, 1], F16),
        "wproj": ([2, 1, 128, 256], F16),
        "wct1": ([2, 1, 128, 32], F32),
        "ct1b": ([32, 1], F32),
        "wct2": ([1, 1, 32, 256], F32),
        "ct2b": ([2, 128, 1], F32),
        "ident": ([128, 128], F16),
        "lng": ([32, 1], F32),
        "lnb": ([32, 1], F32),
    }
    ap = {
        k: nc.dram_tensor(k, shp, dt, kind="ExternalInput").ap()
        for k, (shp, dt) in shapes.items()
    }
    ap["out"] = nc.dram_tensor("out", [BPC, C, H2, W2], F32, kind="ExternalOutput").ap()
    ap["e_scr"] = nc.dram_tensor("e_scr", [1, 4736], BF16, kind="Internal").ap()
    with tile.TileContext(nc) as tc:
        _emit(nc, tc, ap)
    return nc


_CACHED_NC = {}


def _install_trace_hook():
    """The image's antenv lacks axon_hooks; shim it and register the boot's
    ctypes NTFF hook so trace=True works.  Also neutralize the S3 artifact
    upload (no bucket access here)."""
    import types
    import antenv

    if "antenv.axon_hooks" not in sys.modules:
        mod = types.ModuleType("antenv.axon_hooks")
        mod._hook = None
        def set_axon_ntff_profile_hook(h):
            mod._hook = h
        def get_axon_ntff_profile_hook():
            return mod._hook
        mod.set_axon_ntff_profile_hook = set_axon_ntff_profile_hook
        mod.get_axon_ntff_profile_hook = get_axon_ntff_profile_hook
        sys.modules["antenv.axon_hooks"] = mod
        antenv.axon_hooks = mod
        from trn_agent_boot.trn_boot import _ntff_profile_via_ctypes
        mod.set_axon_ntff_profile_hook(
            _ntff_profile_via_ctypes("/opt/axon/libaxon_pjrt.so")
        )
        bass_utils.upload_artifacts = lambda tmpdir: tmpdir


def run(inputs, debug=False, trace=False):
    if trace:
        _install_trace_hook()
    if "nc" not in _CACHED_NC:
        _CACHED_NC["nc"] = build()
    nc = _CACHED_NC["nc"]
    d = _prep_inputs(inputs)
    x_f16 = np.asarray(inputs["x"], np.float32).astype(np.float16)
    in_maps = []
    for c in range(N_CORES):
        m = dict(d)
        m["x"] = np.ascontiguousarray(x_f16[c * BPC : (c + 1) * BPC])
        in_maps.append(m)
    res = bass_utils.run_bass_kernel_spmd(
        nc, in_maps, core_ids=list(range(N_CORES)), trace=trace
    )
    out = np.concatenate([res.results[c]["out"] for c in range(N_CORES)], axis=0)
    return out, res


def kernel(**inputs):
    out, _ = run(inputs)
    return out


# revision 13
# speedup vs baseline: 1.1080x; 1.0185x over previous
"""Trainium2 Bass kernel for nn_Dwtpool (dense_cnn).

Reference graph (per image, C=256, 128x128 input):
  p    = maxpool2x2(x)                          -> [256, 64, 64]
  r    = ReLU(BN(conv1x1(x, reduce_w)))         -> [ 64,128,128]
  M    = haar_dwt(r) * 2  (stored unscaled)     -> [256, 64, 64]
  q2..q4 = conv{3,5,7}(0.5*M)                   -> [256, 64, 64] each
  qkv  = conv3x3(concat[0.5*M, q1..q4, p])      -> [256, 64, 64]
  att  = softmax_spatial(conv1x1(qkv)); pooled = sum_n ch(qkv)_c,n * att_n
  cw   = ct2(ReLU(LN(ct1(pooled))))             -> [256]
  out  = conv1x1(qkv * cw, proj_w)              -> [256, 64, 64]

Strategy: data-parallel over batch (16 images / 8 cores = 2 per core), fp16
trunk (same PE speed as bf16, 8x less noise; the softmax path amplifies qkv
noise ~3x into the output so 16-bit stays mandatory).  All convs are
tap-accumulated f16 matmuls into PSUM.  Algebraic cuts vs the naive graph:
  * q1 (1x1 conv) is folded into concat-conv group 0 on the host
    (conv3x3(W1, conv1x1(c1, x)) == conv3x3(W1 . c1, x)), removing one conv
    and one concat group.
  * the channel conv never materializes: pooled = sum_n a_n*conv(qkv)[:,n]
    == wch . s where s[i,tap] = sum_n a_n * qkv[i, n+tap], computed on the
    DVE with fused tensor_tensor_reduce, then an 18-matmul matvec.
  * content logits are a f16 matmul on the qkv tiles (replicated onto 8
    partitions so exp/accum runs 8-wide), softmax denominator folded into
    the e-broadcast ones-vector.
Phase-1 (reduce+DWT+maxpool) of image 1 is interleaved at the conv-pass PSUM
boundaries of image 0, and image 0's attention tail + proj run inside image
1's first conv pass, keeping the PE stream dense.
"""
import os
import sys

for _p in ("/opt/trn_rl_repo", os.path.expanduser("~/.axon_site/_ro/trn_rl_repo")):
    if os.path.isdir(_p) and _p not in sys.path:
        sys.path.append(_p)

import numpy as np
import ml_dtypes
from contextlib import ExitStack

import concourse.bass as bass
import concourse.tile as tile
from concourse import mybir
from concourse import bass_utils

BF16 = mybir.dt.bfloat16
F16 = mybir.dt.float16
F32 = mybir.dt.float32
AF = mybir.ActivationFunctionType
ALU = mybir.AluOpType

B, C, H, W = 16, 256, 128, 128
H2, W2 = 64, 64
N_CORES = 8
BPC = B // N_CORES  # images per core
EPS = 1e-5

# ---------------------------------------------------------------------------
# walrus CoreV3 rejects instructions with more than a couple of sync waits;
# Tile's exit drain accumulates one wait per processor used.  Split the waits
# across a chain of drain instructions (sync engine executes them in order).
# ---------------------------------------------------------------------------
import bass_rust as _br
import concourse.tile as _tile_mod

def _split_drain_and_barrier(self, tick_clock, wait_clock):
    nc = self.nc
    drain_inst = nc.sync.drain()
    wait_clock.add_sem_waits(
        drain_inst.ins, _tile_mod.ScopedClock({None: tick_clock.global_clock})
    )
    W_ = list(drain_inst.ins.sync_info.on_wait)
    if len(W_) > 1:
        drain_inst.ins.sync_info.on_wait = W_[:1]
        for i in range(1, len(W_)):
            extra = nc.sync.drain()
            extra.ins.sync_info = _br.SyncInfo(on_wait=W_[i : i + 1], on_update=[])
    nc.all_engine_barrier()
    assert self.sems is not None
    popped = nc._tile_sem_poison_stack.pop()
    assert popped is self._sem_poison
    nc.clear_and_free_semaphores(list(self.sems.allocated().values()))
    nc.all_engine_barrier()

tile.TileContext._drain_and_barrier = _split_drain_and_barrier

# Same hardware limit applies to scheduled body instructions (max 2 sync waits
# per instruction).  Before lowering, move excess waits onto injected NOPs on
# the same engine.
_MAX_W = 1
_orig_lower_ordered = tile.TileContext._lower_ordered_insts

def _lower_with_wait_split(self, ordered):
    for _bb, insts in ordered.items():
        out = []
        for inst in insts:
            si = getattr(inst, "sync_info", None)
            if si is not None and len(si.on_wait) > _MAX_W:
                wl = list(si.on_wait)
                extra, keep = wl[:-_MAX_W], wl[-_MAX_W:]
                si.on_wait = keep
                for i in range(0, len(extra), _MAX_W):
                    nop = mybir.InstNoOp(
                        name=f"{inst.name}-wsplit{i}",
                        sync_info=mybir.SyncInfo(
                            on_wait=extra[i : i + _MAX_W], on_update=[]
                        ),
                        bass_nofuse=True,
                        engine=inst.engine,
                    )
                    out.append(nop)
            out.append(inst)
        insts[:] = out
    return _orig_lower_ordered(self, ordered)

tile.TileContext._lower_ordered_insts = _lower_with_wait_split


# ---------------------------------------------------------------------------
# host-side weight packing
# ---------------------------------------------------------------------------
def _pack_conv(w, scale=1.0):
    """[O, I, K, K] -> [n_kt, K*K, kt_size, O]  (lhsT blocks per ktile/tap)."""
    O, I, K, _ = w.shape
    kt = 128 if I >= 128 else I
    nkt = I // kt
    a = (np.asarray(w, np.float32) * scale).transpose(1, 2, 3, 0)  # [I,K,K,O]
    a = a.reshape(nkt, kt, K, K, O).transpose(0, 2, 3, 1, 4)
    return np.ascontiguousarray(a.reshape(nkt, K * K, kt, O))


def _f16(a):
    return np.asarray(a).astype(np.float16)


def _pack_conv5(w, scale=1.0):
    """[O, I, K, K] -> [n_kt, K*K, n_mt, kt, 128]: per (ktile, tap, mtile)
    contiguous lhsT blocks for the streaming conv passes."""
    a = _pack_conv(w, scale)  # [nkt, KK, kt, O]
    nkt, kk, kt, O = a.shape
    return np.ascontiguousarray(
        a.reshape(nkt, kk, kt, O // 128, 128).transpose(0, 1, 3, 2, 4)
    )


def _prep_inputs(inp):
    """Full problem inputs -> dict of packed host arrays (shared by cores)."""
    d = {}
    # reduce conv: fold BN, duplicate output channels to fill 128 partitions
    sc = np.asarray(inp["bn_g"], np.float32) / np.sqrt(
        np.asarray(inp["bn_var"], np.float32) + EPS
    )
    w_red = np.asarray(inp["reduce_w"], np.float32)[:, :, 0, 0] * sc[:, None]  # [64,256]
    b_red = (
        np.asarray(inp["reduce_b"], np.float32) - np.asarray(inp["bn_mean"], np.float32)
    ) * sc + np.asarray(inp["bn_b"], np.float32)
    w_red2 = np.concatenate([w_red, w_red], axis=0)  # [128, 256]
    d["wred"] = _f16(_pack_conv(w_red2[:, :, None, None]))  # [2,1,128,128]
    d["bred"] = np.concatenate([b_red, b_red])[:, None].astype(np.float32)  # [128,1]

    # DWT-branch convs: input is M = 2*qkv0, so fold the 0.5 into weights
    d["w2"] = _f16(_pack_conv5(inp["conv2_w"], 0.5))
    d["w3"] = _f16(_pack_conv5(inp["conv3_w"], 0.5))
    d["w4"] = _f16(_pack_conv5(inp["conv4_w"], 0.5))
    # concat conv groups [qkv0(=0.5*M), q2, q3, q4, p]; the q1 group is folded
    # into group 0: conv3x3(W1, conv1x1(c1, qkv0)) == conv3x3(W1 . c1, qkv0)
    wcat = np.asarray(inp["conv1x1_w"], np.float32)
    conv1 = np.asarray(inp["conv1_w"], np.float32)[:, :, 0, 0]  # [256,256]
    g0 = wcat[:, 0:256] + np.einsum("ocyx,ci->oiyx", wcat[:, 256:512], conv1)
    packs = [_pack_conv5(g0, 0.5)]
    for g in range(2, 6):
        packs.append(_pack_conv5(wcat[:, g * 256 : (g + 1) * 256]))
    d["wcat"] = _f16(np.concatenate(packs, axis=0))  # [10,9,2,128,128]
    d["wch"] = _f16(_pack_conv5(inp["channel_conv_w"]))  # [2,9,2,128,128]
    wc = np.asarray(inp["conv_w"], np.float32)[0, :, 0, 0]  # [256]
    d["wcont16"] = _f16(wc.reshape(2, 128, 1))  # [2,128,1]
    d["wproj"] = _f16(_pack_conv(inp["proj_w"]))  # [2,1,128,256]
    d["wct1"] = _pack_conv(inp["ct1_w"]).astype(np.float32)  # [2,1,128,32]
    d["ct1b"] = np.asarray(inp["ct1_b"], np.float32)[:, None]  # [32,1]
    d["wct2"] = _pack_conv(inp["ct2_w"]).astype(np.float32)  # [1,1,32,256]
    d["ct2b"] = np.asarray(inp["ct2_b"], np.float32).reshape(2, 128, 1)
    d["ident"] = np.eye(128, dtype=np.float16)
    d["lng"] = np.asarray(inp["ln_g"], np.float32)[:, None]
    d["lnb"] = np.asarray(inp["ln_b"], np.float32)[:, None]
    return d


# ---------------------------------------------------------------------------
# kernel body
# ---------------------------------------------------------------------------
def _emit(nc, tc, ap):
    ctx = ExitStack()
    consts = ctx.enter_context(tc.tile_pool(name="consts", bufs=1))
    acts = ctx.enter_context(tc.tile_pool(name="acts", bufs=1))
    wring = ctx.enter_context(tc.tile_pool(name="wring", bufs=1))
    work = ctx.enter_context(tc.tile_pool(name="work", bufs=1))
    psum = ctx.enter_context(tc.tile_pool(name="psum", bufs=8, space="PSUM"))

    def cst(name, shape, dtype, src):
        t = consts.tile(shape, dtype, tag=name, name=name)
        nc.sync.dma_start(out=t, in_=src)
        return t

    wred = [cst(f"wred{k}", [128, 128], F16, ap["wred"][k, 0]) for k in range(2)]
    bred = cst("bred", [128, 1], F32, ap["bred"])
    wcont16 = [cst(f"wcont16{k}", [128, 1], F16, ap["wcont16"][k]) for k in range(2)]
    wproj = [cst(f"wproj{k}", [128, 256], F16, ap["wproj"][k, 0]) for k in range(2)]
    wct1 = [cst(f"wct1{k}", [128, 32], F32, ap["wct1"][k, 0]) for k in range(2)]
    wct2 = cst("wct2", [32, 256], F32, ap["wct2"][0, 0])
    ct1b = cst("ct1b", [32, 1], F32, ap["ct1b"])
    ct2b = [cst(f"ct2b{k}", [128, 1], F32, ap["ct2b"][k]) for k in range(2)]
    ident = cst("ident", [128, 128], F16, ap["ident"])
    lng = cst("lng", [32, 1], F32, ap["lng"])
    lnb = cst("lnb", [32, 1], F32, ap["lnb"])

    sigma = consts.tile([128, 1], F32, tag="sigma", name="sigma")
    nc.vector.memset(sigma[0:64, :], 1.0)
    nc.vector.memset(sigma[64:128, :], -1.0)
    epsv = consts.tile([32, 1], F32, tag="epsv", name="epsv")
    nc.vector.memset(epsv, EPS)
    onesb = consts.tile([1, 128], BF16, tag="onesb", name="onesb")
    nc.vector.memset(onesb, 1.0)
    onesf = consts.tile([32, 1], F32, tag="onesf", name="onesf")
    nc.vector.memset(onesf, 1.0)
    onesf2 = consts.tile([1, 32], F32, tag="onesf2", name="onesf2")
    nc.vector.memset(onesf2, 1.0)

    # ---- activation buffers (f16).  q0/pb are per-image (phase-1 of image
    # i+1 overlaps image i's conv passes); qb/qkv are shared (WAR deps order
    # them behind the previous image's reads, which is late enough).
    def padbuf(name, hw):
        return acts.tile([128, hw, hw], F16, tag=name, name=name)

    q0 = [[padbuf(f"q0_{im}_{k}", 70) for k in range(2)] for im in range(BPC)]
    pb = [[padbuf(f"p_{im}_{k}", 66) for k in range(2)] for im in range(BPC)]
    qb = [[padbuf(f"q{j}_{k}", 66) for k in range(2)] for j in range(3)]
    qkv_t = [
        acts.tile([128, 4480], F16, tag=f"qkv_{k}", name=f"qkv_{k}") for k in range(2)
    ]
    qkv = [
        t[:, 0:4356].rearrange("p (a b) -> p a b", b=66) for t in qkv_t
    ]
    for t in qkv_t:
        nc.gpsimd.memset(t[:, 4356:4480], 0.0)

    # zero the halo borders once (interior is fully overwritten per image)
    for t in [t_ for im in range(BPC) for t_ in q0[im]]:
        nc.gpsimd.memset(t[:, 0:3, :], 0.0)
        nc.gpsimd.memset(t[:, 67:70, :], 0.0)
        nc.gpsimd.memset(t[:, 3:67, 0:3], 0.0)
        nc.gpsimd.memset(t[:, 3:67, 67:70], 0.0)
    for t in [t_ for im in range(BPC) for t_ in pb[im]] + [
        t_ for j in range(3) for t_ in qb[j]
    ] + qkv:
        nc.gpsimd.memset(t[:, 0:1, :], 0.0)
        nc.gpsimd.memset(t[:, 65:66, :], 0.0)
        nc.gpsimd.memset(t[:, 1:65, 0:1], 0.0)
        nc.gpsimd.memset(t[:, 1:65, 65:66], 0.0)

    # ---- phase 1: stream x -> reduce conv+ReLU -> DWT -> M;  maxpool -> p
    class Ph1:
        def __init__(self, img):
            self.img = img
            self.xts = []
            self.sc = 0

        def emit_dmas(self):
            for sc_ in range(16):
                pair = []
                for k in range(2):
                    xt = work.tile(
                        [128, 8, 128], F16, tag=f"x{k}", bufs=3, name=f"xt{k}"
                    )
                    src = ap["x"][
                        self.img, k * 128 : (k + 1) * 128, sc_ * 8 : sc_ * 8 + 8, :
                    ]
                    nc.sync.dma_start(out=xt[:, 0:4, :], in_=src[:, 0:4, :])
                    nc.sync.dma_start(out=xt[:, 4:8, :], in_=src[:, 4:8, :])
                    pair.append(xt)
                self.xts.append(pair)

        def step(self, n=1):
            for _ in range(n):
                if self.sc < 16:
                    self._sc(self.sc)
                    self.sc += 1

        def _sc(self, sc_):
            xts = self.xts[sc_]
            orow = sc_ * 4  # 8 input rows -> 4 output rows
            rch = work.tile([128, 8, 128], F16, tag="rch", bufs=2, name="rch")
            for sub in range(2):
                ps = psum.tile([128, 4, 128], F32, tag="ps", name="ps_r")
                for k in range(2):
                    nc.tensor.matmul(
                        ps, wred[k], xts[k][:, sub * 4 : sub * 4 + 4, :],
                        start=(k == 0), stop=(k == 1),
                    )
                nc.scalar.activation(
                    out=rch[:, sub * 4 : sub * 4 + 4, :], in_=ps, func=AF.Relu,
                    bias=bred, scale=1.0,
                )
            rv = rch.rearrange("p (a two) (c cp) -> p a two c cp", two=2, cp=2)
            a_, b_ = rv[:, :, 0, :, 0], rv[:, :, 0, :, 1]
            c_, d_ = rv[:, :, 1, :, 0], rv[:, :, 1, :, 1]
            u = work.tile([128, 4, 64], F32, tag="u", bufs=2, name="u")
            v = work.tile([128, 4, 64], F32, tag="v", bufs=2, name="v")
            s_ = work.tile([128, 4, 64], F32, tag="s", bufs=2, name="s_")
            t_ = work.tile([128, 4, 64], F32, tag="t", bufs=2, name="t_")
            nc.vector.tensor_add(u, a_, b_)
            nc.vector.tensor_add(v, c_, d_)
            nc.vector.tensor_sub(s_, a_, b_)
            nc.vector.tensor_sub(t_, c_, d_)
            sv = work.tile([128, 4, 64], F32, tag="sv", bufs=2, name="sv")
            st = work.tile([128, 4, 64], F32, tag="st", bufs=2, name="st")
            # sigma-scale on the Scalar engine to unload DVE
            nc.scalar.activation(out=sv, in_=v, func=AF.Copy, scale=sigma)
            nc.scalar.activation(out=st, in_=t_, func=AF.Copy, scale=sigma)
            myq0 = q0[self.img]
            nc.vector.tensor_add(myq0[0][:, 3 + orow : 7 + orow, 3:67], u, sv)
            nc.vector.tensor_add(myq0[1][:, 3 + orow : 7 + orow, 3:67], s_, st)
            for k in range(2):
                xv = xts[k].rearrange("p (a two) (c cp) -> p a two c cp", two=2, cp=2)
                xa, xb = xv[:, :, 0, :, 0], xv[:, :, 0, :, 1]
                xc, xd = xv[:, :, 1, :, 0], xv[:, :, 1, :, 1]
                m1 = work.tile([128, 4, 64], F16, tag="m1", bufs=2, name="m1")
                m2 = work.tile([128, 4, 64], F16, tag="m2", bufs=2, name="m2")
                nc.vector.tensor_max(m1, xa, xb)
                nc.vector.tensor_max(m2, xc, xd)
                nc.vector.tensor_max(pb[self.img][k][:, 1 + orow : 5 + orow, 1:65], m1, m2)

    # ---- conv pass: accumulate over (ktile, tap) into 8 psum banks
    def conv_pass(wdram, n_k, K, rhs_fn, out_fn, boundary_hook=None):
        for qh in range(2):
            pss = [
                [psum.tile([128, 8, 64], F32, tag="ps", name="ps_c") for _ in range(4)]
                for _ in range(2)
            ]
            for ik in range(n_k):
                for tp in range(K * K):
                    wt = wring.tile([128, 256], F16, tag="wtap", bufs=8, name="wt")
                    nc.sync.dma_start(
                        out=wt, in_=wdram[ik, tp].rearrange("m p c -> p m c")
                    )
                    for mt in range(2):
                        lhsT = wt[:, mt * 128 : (mt + 1) * 128]
                        for ci in range(4):
                            r0 = qh * 32 + ci * 8
                            nc.tensor.matmul(
                                pss[mt][ci], lhsT, rhs_fn(ik, tp, r0),
                                start=(ik == 0 and tp == 0),
                                stop=(ik == n_k - 1 and tp == K * K - 1),
                            )
            for ci in range(4):
                for mt in range(2):
                    out_fn(mt, qh * 32 + ci * 8, pss[mt][ci])
            if boundary_hook is not None:
                boundary_hook()

    def emit_image_passes(img, boundary_hook=None):
        """The four conv passes for one image (q2, q3, q4, concat)."""
        g0t = q0[img]

        for j, K in ((0, 3), (1, 5), (2, 7)):
            base = 3 - (K // 2)
            dst = qb[j]

            def rhs_m(ik, tp, r0, K=K, base=base):
                ky, kx = tp // K, tp % K
                return g0t[ik][
                    :, base + ky + r0 : base + ky + r0 + 8, base + kx : base + kx + 64
                ]

            def wr(mt, r0, ps_, dst=dst):
                if mt == 0:
                    nc.vector.tensor_copy(dst[0][:, 1 + r0 : 9 + r0, 1:65], ps_)
                else:
                    nc.scalar.copy(dst[1][:, 1 + r0 : 9 + r0, 1:65], ps_)

            conv_pass(ap[f"w{j+2}"], 2, K, rhs_m, wr, boundary_hook)

        def rhs_cat(ik, tp, r0):
            g, k = ik // 2, ik % 2
            ky, kx = tp // 3, tp % 3
            if g == 0:
                return g0t[k][:, 2 + ky + r0 : 2 + ky + r0 + 8, 2 + kx : 2 + kx + 64]
            src = pb[img][k] if g == 4 else qb[g - 1][k]
            return src[:, ky + r0 : ky + r0 + 8, kx : kx + 64]

        # concat conv drain: write qkv f16; on the mt=1 chunk run the f16
        # content conv on the fresh qkv tiles, exp onto partition r0//8 of e8
        def wr_cat(mt, r0, ps_):
            if mt == 0:
                nc.vector.tensor_copy(qkv[0][:, 1 + r0 : 9 + r0, 1:65], ps_)
                return
            nc.scalar.copy(qkv[1][:, 1 + r0 : 9 + r0, 1:65], ps_)
            ci = r0 // 8
            cp = psum.tile([1, 8, 64], F32, tag="ps", name="cp")
            for k in range(2):
                nc.tensor.matmul(
                    cp, wcont16[k], qkv[k][:, 1 + r0 : 9 + r0, 1:65],
                    start=(k == 0), stop=(k == 1),
                )
            nc.scalar.activation(
                out=e66i[:, 1 + r0 : 9 + r0, 1:65], in_=cp, func=AF.Exp,
                bias=0.0, scale=1.0, accum_out=denc[:, ci : ci + 1],
            )

        conv_pass(ap["wcat"], 10, 3, rhs_cat, wr_cat, boundary_hook)

    # ---- attention tail part A: softmax denominator, then the attention
    # pooling s[i,tap] = sum_n e_n * qkv[i, n+tap] entirely on the PE:
    # transpose qkv spatial->partition tile by tile, contract against 9
    # shifted (unnormalized) e-maps gathered by strided DMA, normalize by
    # 1/den when casting s to f16, transpose s back to channel-major.
    def tail_a(img):
        dent = work.tile([1, 1], F32, tag="dent", name="dent")
        nc.vector.tensor_reduce(
            dent, denc, axis=mybir.AxisListType.X, op=ALU.add
        )
        rden = work.tile([1, 1], F32, tag="rden", name="rden")
        nc.vector.reciprocal(rden, dent)
        rden_bf = work.tile([1, 1], BF16, tag="rden_bf", name="rden_bf")
        nc.vector.tensor_copy(rden_bf, rden)
        rd_ps = psum.tile([128, 1], F32, tag="ps", name="rd_ps")
        nc.tensor.matmul(rd_ps, onesb, rden_bf, start=True, stop=True)
        rdsb = work.tile([128, 1], F32, tag="rdsb", name="rdsb")
        nc.vector.tensor_copy(rdsb, rd_ps)
        # 9 shifted views of the padded e-grid, spatial-on-partitions with
        # m = p*35 + t so every partition reads one contiguous 70B run.
        # (SBUF partition dims can't alias free-dim strides, so bounce the
        # flat e-grid through DRAM scratch and gather from there.)
        nc.sync.dma_start(out=ap["e_scr"], in_=e66_sb)
        aT = work.tile([128, 9, 35], BF16, tag="aT", name="aT")
        for k9 in range(9):
            ky, kx = k9 // 3, k9 % 3
            off = 201 - 66 * ky - kx
            nc.sync.dma_start(
                out=aT[:, k9, :],
                in_=ap["e_scr"][:, off : off + 4480].rearrange(
                    "o (p t) -> (o p) t", t=35
                ),
            )
        s_acc = work.tile([9, 256], F32, tag="s_acc", name="s_acc")
        qv35 = [
            qkv_t[ik].rearrange("p (m t) -> p t m", t=35) for ik in range(2)
        ]
        for t in range(35):
            qpT = work.tile([128, 2, 128], BF16, tag="qpT", bufs=4, name="qpT")
            for ik in range(2):
                tp_ps = psum.tile([128, 128], F16, tag="ps", name="tp_ps")
                nc.tensor.transpose(tp_ps, qv35[ik][:, t, :], ident)
                nc.scalar.copy(qpT[:, ik, :], tp_ps)
            smm = psum.tile([9, 256], F32, tag="ps", name="smm")
            nc.tensor.matmul(
                smm, aT[:, :, t], qpT.rearrange("p a b -> p (a b)"),
                start=True, stop=True,
            )
            if t == 0:
                nc.vector.tensor_copy(s_acc, smm)
            else:
                nc.vector.tensor_add(s_acc, s_acc, smm)
        s_sb = work.tile([9, 256], F16, tag="s_sb", name="s_sb")
        nc.scalar.activation(
            out=s_sb, in_=s_acc, func=AF.Copy, scale=rdsb[0:9, :]
        )
        for ik in range(2):
            st_ps = psum.tile([128, 9], F16, tag="ps", name="st_ps")
            nc.tensor.transpose(
                st_ps, s_sb[:, ik * 128 : (ik + 1) * 128], ident[0:9, 0:9]
            )
            nc.scalar.copy(s16[ik], st_ps)

    # ---- attention tail part B: pooled matvec, channel transform, proj conv
    def tail_b(img):
        pooled_ps = [
            psum.tile([128, 1], F32, tag="ps", name=f"pool_ps{mt}") for mt in range(2)
        ]
        for ik in range(2):
            for tp in range(9):
                wt = wring.tile([128, 256], F16, tag="wtap", bufs=8, name="wtc")
                nc.sync.dma_start(
                    out=wt, in_=ap["wch"][ik, tp].rearrange("m p c -> p m c")
                )
                for mt in range(2):
                    nc.tensor.matmul(
                        pooled_ps[mt], wt[:, mt * 128 : (mt + 1) * 128],
                        s16[ik][:, tp : tp + 1],
                        start=(ik == 0 and tp == 0), stop=(ik == 1 and tp == 8),
                    )
        pooled = []
        for mt in range(2):
            pl = work.tile([128, 1], F32, tag=f"pool{mt}", name="pl")
            nc.vector.tensor_copy(pl, pooled_ps[mt])
            pooled.append(pl)

        # channel transform (tiny, fp32)
        t_ps = psum.tile([32, 1], F32, tag="ps", name="t_ps")
        for k in range(2):
            nc.tensor.matmul(t_ps, wct1[k], pooled[k], start=(k == 0), stop=(k == 1))
        ts2 = work.tile([32, 2], F32, tag="ts2", name="ts2")
        t_sb = ts2[:, 0:1]
        nc.vector.tensor_scalar_add(t_sb, t_ps, ct1b)
        nc.vector.tensor_mul(ts2[:, 1:2], t_sb, t_sb)
        sums_ps = psum.tile([1, 2], F32, tag="ps", name="sums_ps")
        nc.tensor.matmul(sums_ps, onesf, ts2, start=True, stop=True)
        sums_sb = work.tile([1, 2], F32, tag="sums_sb", name="sums_sb")
        nc.vector.tensor_copy(sums_sb, sums_ps)
        bc_ps = psum.tile([32, 2], F32, tag="ps", name="bc_ps")
        nc.tensor.matmul(bc_ps, onesf2, sums_sb, start=True, stop=True)
        mean = work.tile([32, 1], F32, tag="mean", name="mean")
        nc.vector.tensor_scalar_mul(mean, bc_ps[:, 0:1], 1.0 / 32)
        mv = work.tile([32, 1], F32, tag="mv", name="mv")
        nc.vector.tensor_scalar_mul(mv, bc_ps[:, 1:2], 1.0 / 32)
        m2t = work.tile([32, 1], F32, tag="m2t", name="m2t")
        nc.vector.tensor_mul(m2t, mean, mean)
        var = work.tile([32, 1], F32, tag="var", name="var")
        nc.vector.tensor_sub(var, mv, m2t)
        sd = work.tile([32, 1], F32, tag="sd", name="sd")
        nc.scalar.activation(out=sd, in_=var, func=AF.Sqrt, bias=epsv, scale=1.0)
        rsd = work.tile([32, 1], F32, tag="rsd", name="rsd")
        nc.vector.reciprocal(rsd, sd)
        dt_ = work.tile([32, 1], F32, tag="dt", name="dt_")
        nc.vector.tensor_sub(dt_, t_sb, mean)
        tn = work.tile([32, 1], F32, tag="tn", name="tn")
        nc.vector.tensor_mul(tn, dt_, rsd)
        tact = work.tile([32, 1], F32, tag="tact", name="tact")
        nc.scalar.activation(out=tact, in_=tn, func=AF.Relu, bias=lnb, scale=lng)

        projs = []
        for mt in range(2):
            cw_ps = psum.tile([128, 1], F32, tag="ps", name="cw_ps")
            nc.tensor.matmul(
                cw_ps, wct2[:, mt * 128 : (mt + 1) * 128], tact, start=True, stop=True
            )
            cw = work.tile([128, 1], F32, tag=f"cw{mt}", name="cw")
            nc.vector.tensor_scalar_add(cw, cw_ps, ct2b[mt])
            pj = work.tile([128, 256], F16, tag=f"projs{mt}", name="pj")
            nc.vector.tensor_scalar_mul(pj, wproj[mt], cw)
            projs.append(pj)

        # out = proj(qkv * cw)  (cw folded into proj weights)
        for mt in range(2):
            for ci in range(8):
                r0 = ci * 8
                po = psum.tile([128, 8, 64], F32, tag="ps", name="po")
                for k in range(2):
                    nc.tensor.matmul(
                        po,
                        projs[k][:, mt * 128 : (mt + 1) * 128],
                        qkv[k][:, 1 + r0 : 9 + r0, 1:65],
                        start=(k == 0), stop=(k == 1),
                    )
                ost = work.tile([128, 8, 64], F32, tag="ost", bufs=2, name="ost")
                nc.scalar.copy(ost, po)
                nc.sync.dma_start(
                    out=ap["out"][img, mt * 128 : (mt + 1) * 128, r0 : r0 + 8, :],
                    in_=ost,
                )

    # shared tail tiles.  e66_sb: e embedded at offset 134 on a 66-wide,
    # zero-bordered grid (margins make all 9 shifted DMA gathers in-range).
    e66_sb = work.tile([1, 4736], BF16, tag="e66", name="e66")
    nc.gpsimd.memset(e66_sb, 0.0)
    e66i = e66_sb[:, 134 : 134 + 4488].rearrange("o (r c) -> o r c", c=66)
    denc = work.tile([1, 8], F32, tag="denc", name="denc")
    s16 = [work.tile([128, 9], F16, tag=f"s16_{ik}", name="s16") for ik in range(2)]

    # ---- schedule ----
    ph1 = [Ph1(im) for im in range(BPC)]
    ph1[0].emit_dmas()
    ph1[0].step(16)
    ph1[1].emit_dmas()
    # image 0 passes; image 1's phase-1 slots in at the 8 qh boundaries
    emit_image_passes(0, boundary_hook=lambda: ph1[1].step(2))
    tail_a(0)
    tail_b(0)
    emit_image_passes(1)
    tail_a(1)
    tail_b(1)
    ctx.close()


def build():
    nc = bass.Bass("TRN2", target_bir_lowering=False, debug=False)
    shapes = {
        "x": ([BPC, C, H, W], F16),
        "wred": ([2, 1, 128, 128], F16),
        "bred": ([128, 1], F32),
        "w2": ([2, 9, 2, 128, 128], F16),
        "w3": ([2, 25, 2, 128, 128], F16),
        "w4": ([2, 49, 2, 128, 128], F16),
        "wcat": ([10, 9, 2, 128, 128], F16),
        "wch": ([2, 9, 2, 128, 128], F16),
        "wcont16": ([2, 128, 1], F16),
        "wproj": ([2, 1, 128, 256], F16),
        "wct1": ([2, 1, 128, 32], F32),
        "ct1b": ([32, 1], F32),
        "wct2": ([1, 1, 32, 256], F32),
        "ct2b": ([2, 128, 1], F32),
        "ident": ([128, 128], F16),
        "lng": ([32, 1], F32),
        "lnb": ([32, 1], F32),
    }
    ap = {
        k: nc.dram_tensor(k, shp, dt, kind="ExternalInput").ap()
        for k, (shp, dt) in shapes.items()
    }
    ap["out"] = nc.dram_tensor("out", [BPC, C, H2, W2], F32, kind="ExternalOutput").ap()
    ap["e_scr"] = nc.dram_tensor("e_scr", [1, 4736], BF16, kind="Internal").ap()
    with tile.TileContext(nc) as tc:
        _emit(nc, tc, ap)
    return nc


_CACHED_NC = {}


def _install_trace_hook():
    """The image's antenv lacks axon_hooks; shim it and register the boot's
    ctypes NTFF hook so trace=True works.  Also neutralize the S3 artifact
    upload (no bucket access here)."""
    import types
    import antenv

    if "antenv.axon_hooks" not in sys.modules:
        mod = types.ModuleType("antenv.axon_hooks")
        mod._hook = None
        def set_axon_ntff_profile_hook(h):
            mod._hook = h
        def get_axon_ntff_profile_hook():
            return mod._hook
        mod.set_axon_ntff_profile_hook = set_axon_ntff_profile_hook
        mod.get_axon_ntff_profile_hook = get_axon_ntff_profile_hook
        sys.modules["antenv.axon_hooks"] = mod
        antenv.axon_hooks = mod
        from trn_agent_boot.trn_boot import _ntff_profile_via_ctypes
        mod.set_axon_ntff_profile_hook(
            _ntff_profile_via_ctypes("/opt/axon/libaxon_pjrt.so")
        )
        bass_utils.upload_artifacts = lambda tmpdir: tmpdir


def run(inputs, debug=False, trace=False):
    if trace:
        _install_trace_hook()
    if "nc" not in _CACHED_NC:
        _CACHED_NC["nc"] = build()
    nc = _CACHED_NC["nc"]
    d = _prep_inputs(inputs)
    x_f16 = np.asarray(inputs["x"], np.float32).astype(np.float16)
    in_maps = []
    for c in range(N_CORES):
        m = dict(d)
        m["x"] = np.ascontiguousarray(x_f16[c * BPC : (c + 1) * BPC])
        in_maps.append(m)
    res = bass_utils.run_bass_kernel_spmd(
        nc, in_maps, core_ids=list(range(N_CORES)), trace=trace
    )
    out = np.concatenate([res.results[c]["out"] for c in range(N_CORES)], axis=0)
    return out, res


def kernel(**inputs):
    out, _ = run(inputs)
    return out
